# revision 1
# baseline (speedup 1.0000x reference)
"""GAT (2-layer, 4-head then 1-head) on 8 Trainium2 NeuronCores.

Strategy
--------
- Nodes are permuted: globally degree-sorted, dealt round-robin to 8 cores
  (edge balance + nearly-identical degree profiles per core), then each
  core's nodes are laid out in 128-node dst tiles. Tiles are degree-uniform,
  so per-dst edge lists pad to the tile max with tiny waste.
- Edges land in a "slot grid" [128 dst x K slots] per tile: slot-chunk c is
  128 edges whose partition IS the dst row. The aggregation matmul then has
  an identity stationary operand (no per-chunk one-hot masks at all).
- Per-edge messages are fetched with dma_gather (int16 indices). The node
  table is split at the core-5/6 row boundary so both halves fit in int16
  (rows 0..30720 via table A view, rows 30721.. via offset view). Sentinel
  rows (0 and last) have attention logits of -1e30 so padded slots get
  weight exp(-inf) = 0 and contribute nothing, including to the softmax
  denominator.
- Softmax is computed without the max-subtraction (values are O(1); the
  normalization cancels exactly): w_e = exp(leakyrelu(x)) = max(exp(x),
  exp(0.2 x)), accumulated per dst in PSUM along with the denominator, and
  divided once per node.
- Layer outputs are transformed (W2 / W_out) per tile; the layer-2 node
  table is AllGather'd across cores between layers (as is the layer-1
  table after the sharded x @ W1 phase).
"""

import numpy as np

import concourse.bacc as bacc
import concourse.mybir as mybir
import concourse.tile as tile
from concourse.bass_utils import run_bass_kernel_spmd

F32 = mybir.dt.float32
BF16 = mybir.dt.float16  # NB: fp16 (renamed var kept)
I16 = mybir.dt.int16

IN_CH = 128
HID = 32
HEADS = 4
OUT_CH = 112
NEG_SLOPE = 0.2
NEG_BIG = -60000.0

# Layer-1 table row: [h(128) | al_src(4) | al_dst(4) | one(1) | pad] = 256 bf16 (512B)
T1_COLS = 256
T1_USED = 141  # 132 (4x[h|1]) + 8 al
# Layer-2 table row: [z(32) | as2(1) | ad2(1) | one(1) | pad] = 128 bf16 (256B)
T2_COLS = 128
T2_USED = 35

N_CORES = 8
GCAP = 56


def _prep(x, edge_index, W1, a_src1, a_dst1, b1, W2, a_src2, a_dst2, b2, W_out, b_out):
    """Host-side graph preprocessing. Returns (meta, per-core inputs)."""
    N = x.shape[0]
    E = edge_index.shape[1]
    per_core = -(-N // (N_CORES * 128)) * 128
    n_pad = per_core * N_CORES
    NT = per_core // 128
    nrows = n_pad + 2  # + 2 sentinel rows
    b_base = 1 + 6 * per_core  # first table row owned by core 6
    assert b_base - 1 <= 32767 and nrows - b_base <= 32767

    src = np.concatenate([edge_index[0], np.arange(N, dtype=np.int64)])
    dst = np.concatenate([edge_index[1], np.arange(N, dtype=np.int64)])
    E2 = src.shape[0]

    deg = np.bincount(dst, minlength=n_pad)
    order = np.argsort(deg, kind="stable")  # ascending degree, pads first
    # deal round-robin: global rank i -> core i%8, position i//8
    rank = np.empty(n_pad, np.int64)
    rank[order] = np.arange(n_pad)
    core_of = rank % N_CORES
    pos_in_core = rank // N_CORES
    grow = core_of * per_core + pos_in_core      # global row-1 (0-based over n_pad)
    trow = 1 + grow                               # table row of each node
    # perm_rows[g] = node sitting at global row g
    perm_rows = np.empty(n_pad, np.int64)
    perm_rows[grow] = np.arange(n_pad)

    sr = trow[src]
    dr = grow[dst]
    grp = (sr >= b_base).astype(np.int64)  # 0 = A half, 1 = B half

    # sort edges by (dst row, group); compute slot index within each run
    eorder = np.lexsort((grp, dr))
    dr_s = dr[eorder]
    sr_s = sr[eorder]
    g_s = grp[eorder]
    key = dr_s * 2 + g_s
    newrun = np.empty(E2, bool)
    newrun[0] = True
    newrun[1:] = key[1:] != key[:-1]
    run_id = np.cumsum(newrun) - 1
    run_start = np.flatnonzero(newrun)
    slot = np.arange(E2) - run_start[run_id]

    a_cnt = np.bincount(dr_s[g_s == 0], minlength=n_pad)
    b_cnt = np.bincount(dr_s[g_s == 1], minlength=n_pad)
    # per-(core,tile) maxima, unified across cores
    Ka = a_cnt.reshape(N_CORES, NT, 128).max(axis=(0, 2))
    Kb = b_cnt.reshape(N_CORES, NT, 128).max(axis=(0, 2))
    Kt = Ka + Kb

    # adaptive groups: consecutive tiles, sum of slots <= GCAP
    groups = []
    t = 0
    while t < NT:
        e = t
        tot = 0
        while e < NT and (e == t or tot + Ka[e] + Kb[e] <= GCAP):
            tot += Ka[e] + Kb[e]
            e += 1
        groups.append((t, e))
        t = e
    base_a = np.zeros(NT, np.int64)
    base_b = np.zeros(NT, np.int64)
    ga_base = []
    gb_base = []
    off = 0
    for (t0, t1) in groups:
        ga_base.append(off)
        for t in range(t0, t1):
            base_a[t] = off
            off += 128 * Ka[t]
        gb_base.append(off)
        for t in range(t0, t1):
            base_b[t] = off
            off += 128 * Kb[t]
    totidx = off
    sent_b_local = nrows - 1 - b_base

    # default stream = sentinels
    default = np.zeros(totidx, np.int16)
    for t in range(NT):
        default[base_a[t]:base_a[t] + 128 * Ka[t]] = 0
        default[base_b[t]:base_b[t] + 128 * Kb[t]] = sent_b_local
    streams = np.tile(default, (N_CORES, 1))

    e_core = dr_s // per_core
    loc = dr_s % per_core
    tl = loc // 128
    p = loc % 128
    posA = base_a[tl] + slot * 128 + p
    posB = base_b[tl] + slot * 128 + p
    pos = np.where(g_s == 0, posA, posB)
    val = np.where(g_s == 0, sr_s, sr_s - b_base).astype(np.int16)
    streams[e_core, pos] = val

    # wrap for dma_gather: wrapped[p, j] = flat[j*16 + p%16]
    assert totidx % 16 == 0
    idx_wrapped = np.empty((N_CORES, 128, totidx // 16), np.int16)
    for c in range(N_CORES):
        w16 = streams[c].reshape(-1, 16).T  # [16, totidx/16]
        idx_wrapped[c] = np.tile(w16, (8, 1))

    # x slices (table-row order per core)
    xp = np.zeros((n_pad, IN_CH), np.float32)
    xp[:N] = np.asarray(x, np.float32)
    x_slices = np.empty((N_CORES, IN_CH, per_core), np.float32)
    for c in range(N_CORES):
        nodes = perm_rows[c * per_core:(c + 1) * per_core]
        x_slices[c] = xp[nodes].T

    # weight packs
    W1 = np.asarray(W1, np.float32)
    Bsrc = np.zeros((HEADS * HID, HEADS), np.float32)
    Bdst = np.zeros((HEADS * HID, HEADS), np.float32)
    for h in range(HEADS):
        Bsrc[h * HID:(h + 1) * HID, h] = np.asarray(a_src1[h], np.float32)
        Bdst[h * HID:(h + 1) * HID, h] = np.asarray(a_dst1[h], np.float32)
    W1cols = []
    for h in range(HEADS):
        W1cols.append(W1[:, h * HID:(h + 1) * HID])
        W1cols.append(np.zeros((IN_CH, 1), np.float32))  # ones-slot
    W1big = np.concatenate(W1cols + [W1 @ Bsrc, W1 @ Bdst], axis=1)  # [128, 140]
    W2 = np.asarray(W2, np.float32)
    W2big = np.concatenate(
        [W2, W2 @ np.asarray(a_src2, np.float32).T, W2 @ np.asarray(a_dst2, np.float32).T],
        axis=1,
    )  # [128, 34]
    b1v = np.asarray(b1, np.float32).reshape(HEADS, HID)
    b1i = np.zeros((HEADS, HID + 1), np.float32)
    b1i[:, :HID] = b1v
    b1_rep = np.tile(b1i.reshape(1, -1), (128, 1))                            # [128,132]
    b2_rep = np.zeros((128, HID + 2), np.float32)
    b2_rep[:, :HID] = np.asarray(b2, np.float32)[None, :]
    bout_rep = np.tile(np.asarray(b_out, np.float32)[None, :], (128, 1))     # [128,112]
    ident = np.eye(128, dtype=np.float32)

    bf16 = np.float16
    sent1 = np.zeros((1, 144), bf16)
    sent1[0, 132:140] = NEG_BIG
    sent2 = np.zeros((1, 36), bf16)
    sent2[0, 32:34] = NEG_BIG

    meta = dict(
        N=N, E2=E2, n_pad=n_pad, per_core=per_core, NT=NT, nrows=nrows,
        b_base=b_base, Ka=Ka.tolist(), Kb=Kb.tolist(),
        base_a=base_a.tolist(), base_b=base_b.tolist(), totidx=totidx,
        ga_base=ga_base, gb_base=gb_base, groups=groups,
        perm_rows=perm_rows,
    )
    shared = dict(
        W1big=W1big, W2big=W2big.astype(bf16), Wout=np.asarray(W_out, np.float32).astype(bf16),
        b1_rep=b1_rep, b2_rep=b2_rep, bout_rep=bout_rep, ident=ident.astype(bf16),
        sent1=sent1, sent2=sent2,
    )
    in_maps = []
    for c in range(N_CORES):
        m = dict(shared)
        m["x_slice"] = np.ascontiguousarray(x_slices[c])
        m["idx_flat"] = np.ascontiguousarray(idx_wrapped[c])
        in_maps.append(m)
    return meta, in_maps


def _build(meta):
    per_core, NT, nrows, b_base = meta["per_core"], meta["NT"], meta["nrows"], meta["b_base"]
    Ka, Kb = meta["Ka"], meta["Kb"]
    base_a, base_b, totidx = meta["base_a"], meta["base_b"], meta["totidx"]
    ga_base, gb_base, groups = meta["ga_base"], meta["gb_base"], meta["groups"]

    nc = bacc.Bacc("TRN2", num_devices=N_CORES, num_swdge_queues=4,
                   dynamic_dma_scratch_size=65536)

    x_slice = nc.dram_tensor("x_slice", [IN_CH, per_core], F32, kind="ExternalInput")
    idx_flat = nc.dram_tensor("idx_flat", [128, totidx // 16], I16, kind="ExternalInput")
    W1big_d = nc.dram_tensor("W1big", [128, 140], F32, kind="ExternalInput")
    W2big_d = nc.dram_tensor("W2big", [128, HID + 2], BF16, kind="ExternalInput")
    Wout_d = nc.dram_tensor("Wout", [HID, OUT_CH], BF16, kind="ExternalInput")
    b1_d = nc.dram_tensor("b1_rep", [128, 132], F32, kind="ExternalInput")
    b2_d = nc.dram_tensor("b2_rep", [128, HID + 2], F32, kind="ExternalInput")
    bout_d = nc.dram_tensor("bout_rep", [128, OUT_CH], F32, kind="ExternalInput")
    ident_d = nc.dram_tensor("ident", [128, 128], BF16, kind="ExternalInput")
    sent1_d = nc.dram_tensor("sent1", [1, 144], BF16, kind="ExternalInput")
    sent2_d = nc.dram_tensor("sent2", [1, 36], BF16, kind="ExternalInput")

    T1_own = nc.dram_tensor("T1_own", [per_core, T1_COLS], BF16, kind="Internal")
    T1d = nc.dram_tensor("T1d", [per_core, 8], BF16, kind="Internal")
    T2d = nc.dram_tensor("T2d", [per_core, 2], BF16, kind="Internal")
    T1_sh = nc.dram_tensor("T1_sh", [nrows, T1_COLS], BF16, kind="Internal", addr_space="Shared")
    T2_own = nc.dram_tensor("T2_own", [per_core, T2_COLS], BF16, kind="Internal")
    T2_sh = nc.dram_tensor("T2_sh", [nrows, T2_COLS], BF16, kind="Internal", addr_space="Shared")
    out_d = nc.dram_tensor("out", [per_core, OUT_CH], F32, kind="ExternalOutput")

    rgroups = [list(range(N_CORES))]
    qctr = [0]

    def qn():
        q = qctr[0] % 4
        qctr[0] += 1
        return q

    with tile.TileContext(nc) as tc:
        with (
            tc.tile_pool(name="const", bufs=1) as cp,
            tc.tile_pool(name="xa", bufs=2) as xap,
            tc.tile_pool(name="stage", bufs=3) as sp,
            tc.tile_pool(name="g1", bufs=2) as g1p,
            tc.tile_pool(name="g2", bufs=2) as g2p,
            tc.tile_pool(name="small", bufs=4) as smp,
            tc.tile_pool(name="rhs", bufs=2) as rp,
            tc.tile_pool(name="epi", bufs=3) as ep,
            tc.tile_pool(name="psa", bufs=3, space="PSUM") as ppa,
            tc.tile_pool(name="psm", bufs=3, space="PSUM") as ppm,
            tc.tile_pool(name="psy", bufs=2, space="PSUM") as ppy,
        ):
            # ---- consts to SBUF
            W1big = cp.tile([128, 140], F32)
            nc.sync.dma_start(out=W1big[:], in_=W1big_d[:])
            W2big = cp.tile([128, HID + 2], BF16)
            nc.sync.dma_start(out=W2big[:], in_=W2big_d[:])
            Wout = cp.tile([HID, OUT_CH], BF16)
            nc.sync.dma_start(out=Wout[:], in_=Wout_d[:])
            b1r = cp.tile([128, 132], F32)
            nc.sync.dma_start(out=b1r[:], in_=b1_d[:])
            b2r = cp.tile([128, HID + 2], F32)
            nc.sync.dma_start(out=b2r[:], in_=b2_d[:])
            boutr = cp.tile([128, OUT_CH], F32)
            nc.sync.dma_start(out=boutr[:], in_=bout_d[:])
            ident = cp.tile([128, 128], BF16)
            nc.sync.dma_start(out=ident[:], in_=ident_d[:])
            idxs = cp.tile([128, totidx // 16], I16)
            nc.sync.dma_start(out=idxs[:], in_=idx_flat[:])

            # ---- phase A: own node tiles -> T1_own
            for t in range(NT):
                xa = xap.tile([128, 128], F32)
                nc.sync.dma_start(out=xa[:], in_=x_slice[:, t * 128:(t + 1) * 128])
                ps = ppa.tile([128, 140], F32, tag="agg")
                nc.tensor.matmul(out=ps[:], lhsT=xa[:], rhs=W1big[:], start=True, stop=True)
                hb = sp.tile([128, T1_COLS], BF16)
                nc.vector.tensor_copy(out=hb[:, 0:140], in_=ps[:])
                hb_v = hb[:, 0:132].rearrange("p (h j) -> p h j", h=HEADS)
                nc.vector.memset(hb_v[:, :, HID:HID + 1], 1.0)
                nc.vector.memset(hb[:, 140:T1_COLS], 0.0)
                nc.sync.dma_start(out=T1_own[t * 128:(t + 1) * 128, :], in_=hb[:])
                hd = sp.tile([128, 8], BF16, tag="hd")
                nc.vector.tensor_copy(out=hd[:], in_=ps[:, 132:140])
                nc.sync.dma_start(out=T1d[t * 128:(t + 1) * 128, :], in_=hd[:])

            # ---- allgather T1 + sentinel pokes
            nc.gpsimd.collective_compute(
                "AllGather", mybir.AluOpType.bypass, replica_groups=rgroups,
                ins=[T1_own[:]], outs=[T1_sh[1:1 + N_CORES * per_core, :]],
            )
            s1 = cp.tile([1, 144], BF16)
            nc.sync.dma_start(out=s1[:], in_=sent1_d[:])
            nc.sync.dma_start(out=T1_sh[0:1, 0:144], in_=s1[:])
            nc.sync.dma_start(out=T1_sh[nrows - 1:nrows, 0:144], in_=s1[:])

            # ---- layer 1 edge phase (grouped super-gathers)
            for gi, (t0, t1) in enumerate(groups):
                tiles = list(range(t0, t1))
                SA = sum(Ka[t] for t in tiles)
                SB = sum(Kb[t] for t in tiles)
                if SA + SB == 0:
                    continue
                GA = g1p.tile([128, max(SA, 1), T1_COLS], BF16, tag="GA")
                if SA:
                    o = ga_base[gi]
                    nc.gpsimd.dma_gather(
                        GA[:, 0:SA, :], T1_sh[:], idxs[:, o // 16: o // 16 + SA * 8],
                        128 * SA, 128 * SA, T1_COLS,
                        queue_num=qn(), single_packet=False)
                GB = g1p.tile([128, max(SB, 1), T1_COLS], BF16, tag="GB")
                if SB:
                    o = gb_base[gi]
                    nc.gpsimd.dma_gather(
                        GB[:, 0:SB, :], T1_sh[b_base:nrows, :], idxs[:, o // 16: o // 16 + SB * 8],
                        128 * SB, 128 * SB, T1_COLS,
                        queue_num=qn(), single_packet=False)
                for t in tiles:
                    ka, kb = Ka[t], Kb[t]
                    kt = ka + kb
                    if kt == 0:
                        continue
                    aoff = (base_a[t] - ga_base[gi]) // 128
                    boff = (base_b[t] - gb_base[gi]) // 128
                    al8 = smp.tile([128, 8], BF16, tag="al8")
                    nc.sync.dma_start(out=al8[:], in_=T1d[t * 128:(t + 1) * 128, :])
                    ps = ppa.tile([128, HEADS * (HID + 1)], F32, tag="agg")
                    rhs = rp.tile([128, kt, HEADS * (HID + 1)], BF16, tag="rhs1")
                    rhs_v = rhs[:].rearrange("p k (h j) -> p k h j", h=HEADS)
                    for (G, goff, nk, ro) in ((GA, aoff, ka, 0), (GB, boff, kb, ka)):
                        if nk == 0:
                            continue
                        Gs = G[:, goff:goff + nk, :]
                        xl = smp.tile([128, nk, HEADS], BF16, tag="xl")
                        nc.vector.tensor_tensor(
                            out=xl[:], in0=Gs[:, :, 132:136],
                            in1=al8[:, None, 4:8].to_broadcast([128, nk, HEADS]),
                            op=mybir.AluOpType.add)
                        e1 = smp.tile([128, nk, HEADS], BF16, tag="e1")
                        nc.scalar.activation(e1[:], xl[:], mybir.ActivationFunctionType.Exp)
                        e2 = smp.tile([128, nk, HEADS], BF16, tag="e2")
                        nc.scalar.activation(e2[:], xl[:], mybir.ActivationFunctionType.Exp, scale=NEG_SLOPE)
                        w = smp.tile([128, nk, HEADS], BF16, tag="w")
                        nc.vector.tensor_tensor(out=w[:], in0=e1[:], in1=e2[:], op=mybir.AluOpType.max)
                        nc.vector.tensor_tensor(
                            out=rhs_v[:, ro:ro + nk, :, :],
                            in0=Gs[:, :, 0:132].rearrange("p k (h j) -> p k h j", h=HEADS),
                            in1=w[:, :, :, None].to_broadcast([128, nk, HEADS, HID + 1]),
                            op=mybir.AluOpType.mult)
                    for cch in range(kt):
                        nc.tensor.matmul(out=ps[:], lhsT=ident[:], rhs=rhs[:, cch, :],
                                         start=(cch == 0), stop=(cch == kt - 1))
                    # epilogue: divide, +b1, ELU
                    ps_v = ps[:].rearrange("p (h j) -> p h j", h=HEADS)
                    rec = smp.tile([128, HEADS], F32, tag="rec")
                    nc.vector.reciprocal(out=rec[:], in_=ps_v[:, :, HID])
                    y1 = ppy.tile([128, 128], F32, tag="y")
                    nc.vector.tensor_tensor(
                        out=y1[:].rearrange("p (h j) -> p h j", h=HEADS),
                        in0=ps_v[:, :, 0:HID],
                        in1=rec[:, :, None].to_broadcast([128, HEADS, HID]),
                        op=mybir.AluOpType.mult)
                    nc.vector.tensor_tensor(
                        out=y1[:].rearrange("p (h j) -> p h j", h=HEADS),
                        in0=y1[:].rearrange("p (h j) -> p h j", h=HEADS),
                        in1=b1r[:].rearrange("p (h j) -> p h j", h=HEADS)[:, :, 0:HID],
                        op=mybir.AluOpType.add)
                    m1 = ep.tile([128, 128], F32, tag="m1")
                    nc.vector.tensor_scalar(out=m1[:], in0=y1[:], scalar1=0.0, scalar2=None,
                                            op0=mybir.AluOpType.min)
                    eE = ep.tile([128, 128], F32, tag="eE")
                    nc.scalar.activation(eE[:], m1[:], mybir.ActivationFunctionType.Exp)
                    r1 = ep.tile([128, 128], F32, tag="r1")
                    nc.vector.tensor_scalar(out=r1[:], in0=y1[:], scalar1=0.0, scalar2=-1.0,
                                            op0=mybir.AluOpType.max, op1=mybir.AluOpType.add)
                    h2 = ep.tile([128, 128], BF16, tag="h2")
                    nc.vector.tensor_tensor(out=h2[:], in0=eE[:], in1=r1[:], op=mybir.AluOpType.add)
                    # transpose h2, z = h2 @ W2big
                    pt = ppm.tile([128, 128], BF16, tag="misc")
                    nc.tensor.transpose(out=pt[:], in_=h2[:], identity=ident[:])
                    h2T = ep.tile([128, 128], BF16, tag="h2T")
                    nc.vector.tensor_copy(out=h2T[:], in_=pt[:])
                    psz = ppm.tile([128, HID + 2], F32, tag="misc")
                    nc.tensor.matmul(out=psz[:], lhsT=h2T[:], rhs=W2big[:], start=True, stop=True)
                    t2b = sp.tile([128, T2_COLS], BF16, tag="t2b")
                    nc.vector.tensor_tensor(out=t2b[:, 0:HID + 2], in0=psz[:], in1=b2r[:],
                                            op=mybir.AluOpType.add)
                    nc.vector.memset(t2b[:, HID + 2:HID + 3], 1.0)
                    nc.vector.memset(t2b[:, HID + 3:T2_COLS], 0.0)
                    nc.sync.dma_start(out=T2_own[t * 128:(t + 1) * 128, :], in_=t2b[:])
                    td2 = sp.tile([128, 2], BF16, tag="td2")
                    nc.vector.tensor_tensor(out=td2[:], in0=psz[:, 32:34], in1=b2r[:, 32:34],
                                            op=mybir.AluOpType.add)
                    nc.sync.dma_start(out=T2d[t * 128:(t + 1) * 128, :], in_=td2[:])

            # ---- allgather T2 + sentinel pokes
            nc.gpsimd.collective_compute(
                "AllGather", mybir.AluOpType.bypass, replica_groups=rgroups,
                ins=[T2_own[:]], outs=[T2_sh[1:1 + N_CORES * per_core, :]],
            )
            s2 = cp.tile([1, 36], BF16)
            nc.sync.dma_start(out=s2[:], in_=sent2_d[:])
            nc.sync.dma_start(out=T2_sh[0:1, 0:36], in_=s2[:])
            nc.sync.dma_start(out=T2_sh[nrows - 1:nrows, 0:36], in_=s2[:])

            # ---- layer 2 edge phase + output (grouped super-gathers)
            for gi, (t0, t1) in enumerate(groups):
                tiles2 = list(range(t0, t1))
                SA = sum(Ka[t] for t in tiles2)
                SB = sum(Kb[t] for t in tiles2)
                if SA + SB == 0:
                    continue
                GA2 = g2p.tile([128, max(SA, 1), T2_COLS], BF16, tag="GA2")
                if SA:
                    o = ga_base[gi]
                    nc.gpsimd.dma_gather(
                        GA2[:, 0:SA, :], T2_sh[:], idxs[:, o // 16: o // 16 + SA * 8],
                        128 * SA, 128 * SA, T2_COLS,
                        queue_num=qn(), single_packet=False)
                GB2 = g2p.tile([128, max(SB, 1), T2_COLS], BF16, tag="GB2")
                if SB:
                    o = gb_base[gi]
                    nc.gpsimd.dma_gather(
                        GB2[:, 0:SB, :], T2_sh[b_base:nrows, :], idxs[:, o // 16: o // 16 + SB * 8],
                        128 * SB, 128 * SB, T2_COLS,
                        queue_num=qn(), single_packet=False)
                for t in tiles2:
                    ka, kb = Ka[t], Kb[t]
                    kt = ka + kb
                    if kt == 0:
                        continue
                    aoff = (base_a[t] - ga_base[gi]) // 128
                    boff = (base_b[t] - gb_base[gi]) // 128
                    ad2 = smp.tile([128, 2], BF16, tag="ad2")
                    nc.sync.dma_start(out=ad2[:], in_=T2d[t * 128:(t + 1) * 128, :])
                    ps2 = ppa.tile([128, T2_USED], F32, tag="agg")
                    rhs2 = rp.tile([128, kt, T2_USED], BF16, tag="rhs2")
                    for (G2, goff, nk, ro) in ((GA2, aoff, ka, 0), (GB2, boff, kb, ka)):
                        if nk == 0:
                            continue
                        Gs = G2[:, goff:goff + nk, :]
                        xl2 = smp.tile([128, nk, 1], BF16, tag="xl2")
                        nc.vector.tensor_tensor(
                            out=xl2[:], in0=Gs[:, :, 32:33],
                            in1=ad2[:, None, 1:2].to_broadcast([128, nk, 1]),
                            op=mybir.AluOpType.add)
                        e1b = smp.tile([128, nk, 1], BF16, tag="e1b")
                        nc.scalar.activation(e1b[:], xl2[:], mybir.ActivationFunctionType.Exp)
                        e2b = smp.tile([128, nk, 1], BF16, tag="e2b")
                        nc.scalar.activation(e2b[:], xl2[:], mybir.ActivationFunctionType.Exp, scale=NEG_SLOPE)
                        w2 = smp.tile([128, nk, 1], BF16, tag="w2")
                        nc.vector.tensor_tensor(out=w2[:], in0=e1b[:], in1=e2b[:], op=mybir.AluOpType.max)
                        nc.vector.tensor_tensor(
                            out=rhs2[:, ro:ro + nk, :],
                            in0=Gs[:, :, 0:T2_USED],
                            in1=w2[:, :, :].to_broadcast([128, nk, T2_USED]),
                            op=mybir.AluOpType.mult)
                    for cch in range(kt):
                        nc.tensor.matmul(out=ps2[:], lhsT=ident[:], rhs=rhs2[:, cch, :],
                                         start=(cch == 0), stop=(cch == kt - 1))
                    rec2 = smp.tile([128, 1], F32, tag="rec2")
                    nc.vector.reciprocal(out=rec2[:], in_=ps2[:, HID + 2:HID + 3])
                    y2 = ppy.tile([128, HID], F32, tag="y")
                    nc.vector.tensor_tensor(
                        out=y2[:], in0=ps2[:, 0:HID],
                        in1=rec2[:].to_broadcast([128, HID]),
                        op=mybir.AluOpType.mult)
                    m2 = ep.tile([128, HID], F32, tag="m2")
                    nc.vector.tensor_scalar(out=m2[:], in0=y2[:], scalar1=0.0, scalar2=None,
                                            op0=mybir.AluOpType.min)
                    eE2 = ep.tile([128, HID], F32, tag="eE2")
                    nc.scalar.activation(eE2[:], m2[:], mybir.ActivationFunctionType.Exp)
                    r2 = ep.tile([128, HID], F32, tag="r2")
                    nc.vector.tensor_scalar(out=r2[:], in0=y2[:], scalar1=0.0, scalar2=-1.0,
                                            op0=mybir.AluOpType.max, op1=mybir.AluOpType.add)
                    h3 = ep.tile([128, HID], BF16, tag="h3")
                    nc.vector.tensor_tensor(out=h3[:], in0=eE2[:], in1=r2[:], op=mybir.AluOpType.add)
                    pt2 = ppm.tile([128, 128], BF16, tag="misc")
                    nc.tensor.transpose(out=pt2[:HID, :], in_=h3[:], identity=ident[:])
                    h3T = ep.tile([HID, 128], BF16, tag="h3T")
                    nc.vector.tensor_copy(out=h3T[:], in_=pt2[:HID, :])
                    psf = ppm.tile([128, OUT_CH], F32, tag="misc")
                    nc.tensor.matmul(out=psf[:], lhsT=h3T[:], rhs=Wout[:], start=True, stop=True)
                    outf = ep.tile([128, OUT_CH], F32, tag="outf")
                    nc.vector.tensor_tensor(out=outf[:], in0=psf[:], in1=boutr[:],
                                            op=mybir.AluOpType.add)
                    nc.sync.dma_start(out=out_d[t * 128:(t + 1) * 128, :], in_=outf[:])

    nc.compile()
    return nc


def _run(inputs, trace=False):
    meta, in_maps = _prep(**inputs)
    nc = _build(meta)
    res = run_bass_kernel_spmd(nc, in_maps, core_ids=list(range(N_CORES)), trace=trace)
    per_core = meta["per_core"]
    outg = np.concatenate([res.results[c]["out"] for c in range(N_CORES)], axis=0)
    # global row g holds node perm_rows[g]
    out_nodes = np.empty((meta["n_pad"], OUT_CH), np.float32)
    out_nodes[meta["perm_rows"]] = outg
    return out_nodes[:meta["N"]], res


def kernel(**inputs):
    out, _ = _run(inputs, trace=False)
    return out



# revision 13
# speedup vs baseline: 1.9105x; 1.9105x over previous
"""GAT (2-layer, 4-head then 1-head) on 8 Trainium2 NeuronCores.

Strategy
--------
- Nodes are permuted: globally degree-sorted, dealt round-robin to 8 cores
  (edge balance + nearly-identical degree profiles per core), then each
  core's nodes are laid out in 128-node dst tiles. Tiles are degree-uniform,
  so per-dst edge lists pad to the tile max with small waste.
- Edges land in a "slot grid" [128 dst x K slots] per tile: slot-chunk c is
  128 edges whose partition IS the dst row. The aggregation matmul then has
  an identity stationary operand.
- Per-edge messages are fetched with dma_gather (int16 indices) issued as
  prepare_only descriptors + trigger_dma so transfers run async on 4 SWDGE
  queues. The bottleneck is Q7 descriptor generation (~8ns/idx), so the
  index count is minimized:
    * self-loops are NOT gathered: each tile's self contribution is built
      locally as one extra rhs chunk (w_self * [h|1]).
    * the int16 range split uses OVERLAPPING views (A = rows 0..32767 from
      base 0, B = rows 8194..40961 from base 8194). Edges with src row in
      [8194, 32767] can go to either side and are assigned to balance the
      per-tile (Ka, Kb) caps, nearly halving the padding.
- Sentinel rows (0 and last) have attention logits of -60000 so padded
  slots get weight exp(-inf)=0 and contribute nothing.
- Softmax without max-subtraction: w_e = exp(leakyrelu(x)) = max(exp(x),
  exp(0.2 x)), accumulated per dst in PSUM along with the denominator
  (ones-column trick), divided once per node.
- Layer outputs are transformed (W2 / W_out) per tile; node tables are
  AllGather'd across cores between layers.
"""

import numpy as np

import concourse.bacc as bacc
import concourse.mybir as mybir
import concourse.tile as tile
from concourse.bass_utils import run_bass_kernel_spmd

F32 = mybir.dt.float32
BF16 = mybir.dt.float16  # NB: fp16 (renamed var kept)
I16 = mybir.dt.int16

IN_CH = 128
HID = 32
HEADS = 4
OUT_CH = 112
NEG_SLOPE = 0.2
NEG_BIG = -60000.0

# Layer-1 table row: [h(128) | al_src(4) | al_dst(4) | one(1) | pad] = 256 bf16 (512B)
T1_COLS = 256
# Layer-2 table row: [z(32) | as2(1) | ad2(1) | one(1) | pad] = 128 bf16 (256B)
T2_COLS = 128
T2_USED = 35

N_CORES = 8
GCAP = 56


def _prep(x, edge_index, W1, a_src1, a_dst1, b1, W2, a_src2, a_dst2, b2, W_out, b_out):
    """Host-side graph preprocessing. Returns (meta, per-core inputs)."""
    N = x.shape[0]
    per_core = -(-N // (N_CORES * 128)) * 128
    n_pad = per_core * N_CORES
    NT = per_core // 128
    nrows = n_pad + 2  # + 2 sentinel rows
    BB = nrows - 1 - 32767  # first row of the B view; B covers rows BB..nrows-1
    assert BB >= 0 and BB <= 32767

    src = np.asarray(edge_index[0], np.int64)
    dst = np.asarray(edge_index[1], np.int64)
    E2 = src.shape[0]

    deg_s = np.bincount(dst, minlength=n_pad) + 1  # incl self loop, for sorting
    order = np.argsort(deg_s, kind="stable")  # ascending degree, pads first
    rank = np.empty(n_pad, np.int64)
    rank[order] = np.arange(n_pad)
    core_of = rank % N_CORES
    pos_in_core = rank // N_CORES
    grow = core_of * per_core + pos_in_core      # global row-1 (0-based over n_pad)
    trow = 1 + grow                               # table row of each node
    perm_rows = np.empty(n_pad, np.int64)
    perm_rows[grow] = np.arange(n_pad)

    sr = trow[src]
    dr = grow[dst]
    # class: 0 = A-only (row < BB), 1 = flex, 2 = B-only (row > 32767)
    cls = np.where(sr < BB, 0, np.where(sr > 32767, 2, 1)).astype(np.int64)

    deg = np.bincount(dr, minlength=n_pad)          # per dst row, no self
    nA = np.bincount(dr[cls == 0], minlength=n_pad)
    nB = np.bincount(dr[cls == 2], minlength=n_pad)
    tile_of = (np.arange(n_pad) % per_core) // 128

    Ka = np.zeros(NT, np.int64)
    Kb = np.zeros(NT, np.int64)
    for t in range(NT):
        sel = tile_of == t
        ka = nA[sel].max()
        kb = nB[sel].max()
        ka += max(0, deg[sel].max() - ka - kb)  # ensure Ka+Kb >= maxdeg
        Ka[t], Kb[t] = ka, kb
    Kt = Ka + Kb

    # per-dst A-count: a = max(nA, deg - Kb[tile])
    a_of = np.maximum(nA, deg - Kb[tile_of])

    # sort edges by (dst row, class); first a_of[d] edges of each run -> A
    eorder = np.lexsort((cls, dr))
    dr_s = dr[eorder]
    sr_s = sr[eorder]
    newrun = np.empty(E2, bool)
    newrun[0] = True
    newrun[1:] = dr_s[1:] != dr_s[:-1]
    run_start_idx = np.flatnonzero(newrun)
    run_id = np.cumsum(newrun) - 1
    pos_in_run = np.arange(E2) - run_start_idx[run_id]
    in_A = pos_in_run < a_of[dr_s]
    # slot within its side
    slot = np.where(in_A, pos_in_run, pos_in_run - a_of[dr_s])

    # adaptive groups: consecutive tiles, sum of slots <= GCAP
    groups = []
    t = 0
    while t < NT:
        e = t
        tot = 0
        while e < NT and (e == t or tot + Kt[e] <= GCAP):
            tot += Kt[e]
            e += 1
        groups.append((t, e))
        t = e
    base_a = np.zeros(NT, np.int64)
    base_b = np.zeros(NT, np.int64)
    ga_base = []
    gb_base = []
    off = 0
    for (t0, t1) in groups:
        ga_base.append(off)
        for t in range(t0, t1):
            base_a[t] = off
            off += 128 * Ka[t]
        gb_base.append(off)
        for t in range(t0, t1):
            base_b[t] = off
            off += 128 * Kb[t]
    totidx = off

    # default stream = sentinels (A: row 0; B: local 32767 = last row)
    default = np.zeros(totidx, np.int16)
    for t in range(NT):
        default[base_a[t]:base_a[t] + 128 * Ka[t]] = 0
        default[base_b[t]:base_b[t] + 128 * Kb[t]] = 32767
    streams = np.tile(default, (N_CORES, 1))

    e_core = dr_s // per_core
    loc = dr_s % per_core
    tl = loc // 128
    p = loc % 128
    posA = base_a[tl] + slot * 128 + p
    posB = base_b[tl] + slot * 128 + p
    pos = np.where(in_A, posA, posB)
    val = np.where(in_A, sr_s, sr_s - BB).astype(np.int16)
    assert val.min() >= 0
    streams[e_core, pos] = val

    # wrap for dma_gather: wrapped[p, j] = flat[j*16 + p%16]
    assert totidx % 16 == 0
    idx_wrapped = np.empty((N_CORES, 128, totidx // 16), np.int16)
    for c in range(N_CORES):
        w16 = streams[c].reshape(-1, 16).T  # [16, totidx/16]
        idx_wrapped[c] = np.tile(w16, (8, 1))

    # x slices (table-row order per core), fp16 for fast PE loads
    xp = np.zeros((n_pad, IN_CH), np.float32)
    xp[:N] = np.asarray(x, np.float32)
    x_slices = np.empty((N_CORES, IN_CH, per_core), np.float16)
    for c in range(N_CORES):
        nodes = perm_rows[c * per_core:(c + 1) * per_core]
        x_slices[c] = xp[nodes].T.astype(np.float16)

    # weight packs
    W1 = np.asarray(W1, np.float32)
    Bsrc = np.zeros((HEADS * HID, HEADS), np.float32)
    Bdst = np.zeros((HEADS * HID, HEADS), np.float32)
    for h in range(HEADS):
        Bsrc[h * HID:(h + 1) * HID, h] = np.asarray(a_src1[h], np.float32)
        Bdst[h * HID:(h + 1) * HID, h] = np.asarray(a_dst1[h], np.float32)
    W1cols = []
    for h in range(HEADS):
        W1cols.append(W1[:, h * HID:(h + 1) * HID])
        W1cols.append(np.zeros((IN_CH, 1), np.float32))  # ones-slot
    W1big = np.concatenate(W1cols + [W1 @ Bsrc, W1 @ Bdst], axis=1)  # [128, 140]
    W2 = np.asarray(W2, np.float32)
    W2big = np.concatenate(
        [W2, W2 @ np.asarray(a_src2, np.float32).T, W2 @ np.asarray(a_dst2, np.float32).T],
        axis=1,
    )  # [128, 34]
    b1v = np.asarray(b1, np.float32).reshape(HEADS, HID)
    b1i = np.zeros((HEADS, HID + 1), np.float32)
    b1i[:, :HID] = b1v
    b1_rep = np.tile(b1i.reshape(1, -1), (128, 1))                            # [128,132]
    b2_rep = np.zeros((128, HID + 2), np.float32)
    b2_rep[:, :HID] = np.asarray(b2, np.float32)[None, :]
    bout_rep = np.tile(np.asarray(b_out, np.float32)[None, :], (128, 1))     # [128,112]
    ident = np.eye(128, dtype=np.float32)

    bf16 = np.float16
    sent1 = np.zeros((1, 144), bf16)
    sent1[0, 132:140] = NEG_BIG
    sent2 = np.zeros((1, 36), bf16)
    sent2[0, 32:34] = NEG_BIG

    meta = dict(
        N=N, E2=E2, n_pad=n_pad, per_core=per_core, NT=NT, nrows=nrows,
        b_base=BB, Ka=Ka.tolist(), Kb=Kb.tolist(),
        base_a=base_a.tolist(), base_b=base_b.tolist(), totidx=totidx,
        ga_base=ga_base, gb_base=gb_base, groups=groups,
        perm_rows=perm_rows,
    )
    shared = dict(
        W1big=W1big.astype(bf16), W2big=W2big.astype(bf16),
        Wout=np.asarray(W_out, np.float32).astype(bf16),
        b1_rep=b1_rep, b2_rep=b2_rep, bout_rep=bout_rep, ident=ident.astype(bf16),
        sent1=sent1, sent2=sent2,
    )
    in_maps = []
    for c in range(N_CORES):
        m = dict(shared)
        m["x_slice"] = np.ascontiguousarray(x_slices[c])
        m["idx_flat"] = np.ascontiguousarray(idx_wrapped[c])
        in_maps.append(m)
    return meta, in_maps


def _build(meta):
    per_core, NT, nrows, b_base = meta["per_core"], meta["NT"], meta["nrows"], meta["b_base"]
    Ka, Kb = meta["Ka"], meta["Kb"]
    base_a, base_b, totidx = meta["base_a"], meta["base_b"], meta["totidx"]
    ga_base, gb_base, groups = meta["ga_base"], meta["gb_base"], meta["groups"]

    nc = bacc.Bacc("TRN2", num_devices=N_CORES, num_swdge_queues=4,
                   dynamic_dma_scratch_size=65536)

    x_slice = nc.dram_tensor("x_slice", [IN_CH, per_core], BF16, kind="ExternalInput")
    idx_flat = nc.dram_tensor("idx_flat", [128, totidx // 16], I16, kind="ExternalInput")
    W1big_d = nc.dram_tensor("W1big", [128, 140], BF16, kind="ExternalInput")
    W2big_d = nc.dram_tensor("W2big", [128, HID + 2], BF16, kind="ExternalInput")
    Wout_d = nc.dram_tensor("Wout", [HID, OUT_CH], BF16, kind="ExternalInput")
    b1_d = nc.dram_tensor("b1_rep", [128, 132], F32, kind="ExternalInput")
    b2_d = nc.dram_tensor("b2_rep", [128, HID + 2], F32, kind="ExternalInput")
    bout_d = nc.dram_tensor("bout_rep", [128, OUT_CH], F32, kind="ExternalInput")
    ident_d = nc.dram_tensor("ident", [128, 128], BF16, kind="ExternalInput")
    sent1_d = nc.dram_tensor("sent1", [1, 144], BF16, kind="ExternalInput")
    sent2_d = nc.dram_tensor("sent2", [1, 36], BF16, kind="ExternalInput")

    T1_own = nc.dram_tensor("T1_own", [per_core, T1_COLS], BF16, kind="Internal")
    T1_sh = nc.dram_tensor("T1_sh", [nrows, T1_COLS], BF16, kind="Internal", addr_space="Shared")
    T2_own = nc.dram_tensor("T2_own", [per_core, T2_COLS], BF16, kind="Internal")
    T2_sh = nc.dram_tensor("T2_sh", [nrows, T2_COLS], BF16, kind="Internal", addr_space="Shared")
    out_d = nc.dram_tensor("out", [per_core, OUT_CH], F32, kind="ExternalOutput")

    rgroups = [list(range(N_CORES))]
    qctr = [0]
    qsems = [nc.alloc_semaphore(f"gsem{q}") for q in range(4)]

    def qn():
        q = qctr[0] % 4
        qctr[0] += 1
        return q

    import os
    ASYNC = os.environ.get("GAT_SYNC_GATHER", "0") != "1"

    def async_gather(out_ap, table_ap, idx_ap, nidx, cols):
        """Prep descriptors on a rotating SWDGE queue; fire the DMA async."""
        q = qn()
        if ASYNC:
            nc.gpsimd.dma_gather(
                out_ap, table_ap, idx_ap, nidx, nidx, cols,
                prepare_only=True, sem=qsems[q], queue_num=q, single_packet=False)
            nc.gpsimd.trigger_dma(count=None, queue_num=q)
        else:
            nc.gpsimd.dma_gather(
                out_ap, table_ap, idx_ap, nidx, nidx, cols,
                queue_num=q, single_packet=False)

    with tile.TileContext(nc) as tc:
        with (
            tc.tile_pool(name="const", bufs=1) as cp,
            tc.tile_pool(name="xa", bufs=2) as xap,
            tc.tile_pool(name="stage", bufs=3) as sp,
            tc.tile_pool(name="g1", bufs=2) as g1p,
            tc.tile_pool(name="g2", bufs=2) as g2p,
            tc.tile_pool(name="small", bufs=4) as smp,
            tc.tile_pool(name="rhs", bufs=2) as rp,
            tc.tile_pool(name="epi", bufs=3) as ep,
            tc.tile_pool(name="psa", bufs=3, space="PSUM") as ppa,
            tc.tile_pool(name="psm", bufs=3, space="PSUM") as ppm,
            tc.tile_pool(name="psy", bufs=2, space="PSUM") as ppy,
        ):
            # ---- consts to SBUF
            W1big = cp.tile([128, 140], BF16)
            nc.sync.dma_start(out=W1big[:], in_=W1big_d[:])
            W2big = cp.tile([128, HID + 2], BF16)
            nc.sync.dma_start(out=W2big[:], in_=W2big_d[:])
            Wout = cp.tile([HID, OUT_CH], BF16)
            nc.sync.dma_start(out=Wout[:], in_=Wout_d[:])
            b1r = cp.tile([128, 132], F32)
            nc.sync.dma_start(out=b1r[:], in_=b1_d[:])
            b2r = cp.tile([128, HID + 2], F32)
            nc.sync.dma_start(out=b2r[:], in_=b2_d[:])
            boutr = cp.tile([128, OUT_CH], F32)
            nc.sync.dma_start(out=boutr[:], in_=bout_d[:])
            ident = cp.tile([128, 128], BF16)
            nc.sync.dma_start(out=ident[:], in_=ident_d[:])
            idxs = cp.tile([128, totidx // 16], I16)
            nc.sync.dma_start(out=idxs[:], in_=idx_flat[:])
            # SBUF-resident own-node data (no DRAM roundtrip)
            t1d_sb = cp.tile([128, NT * 8], BF16)    # [al_src(4)|al_dst(4)] per tile
            t2d_sb = cp.tile([128, NT * 2], BF16)    # [as2|ad2] per tile
            hball = cp.tile([128, NT * 132], BF16)   # own [h|1]x4 rows per tile
            zball = cp.tile([128, NT * T2_USED], BF16)  # own [z|as2|ad2|1] per tile

            # ---- phase A: own node tiles -> T1_own
            for t in range(NT):
                xa = xap.tile([128, 128], BF16)
                nc.sync.dma_start(out=xa[:], in_=x_slice[:, t * 128:(t + 1) * 128])
                ps = ppa.tile([128, 140], F32, tag="agg")
                nc.tensor.matmul(out=ps[:], lhsT=xa[:], rhs=W1big[:], start=True, stop=True)
                hb = sp.tile([128, T1_COLS], BF16)
                nc.vector.tensor_copy(out=hb[:, 0:140], in_=ps[:])
                hb_v = hb[:, 0:132].rearrange("p (h j) -> p h j", h=HEADS)
                nc.vector.memset(hb_v[:, :, HID:HID + 1], 1.0)
                nc.sync.dma_start(out=T1_own[t * 128:(t + 1) * 128, :], in_=hb[:])
                nc.vector.tensor_copy(out=t1d_sb[:, t * 8:(t + 1) * 8], in_=ps[:, 132:140])
                nc.vector.tensor_copy(out=hball[:, t * 132:(t + 1) * 132], in_=hb[:, 0:132])

            # ---- allgather T1 + sentinel pokes
            nc.gpsimd.collective_compute(
                "AllGather", mybir.AluOpType.bypass, replica_groups=rgroups,
                ins=[T1_own[:]], outs=[T1_sh[1:1 + N_CORES * per_core, :]],
            )
            s1 = cp.tile([1, 144], BF16)
            nc.sync.dma_start(out=s1[:], in_=sent1_d[:])
            nc.sync.dma_start(out=T1_sh[0:1, 0:144], in_=s1[:])
            nc.sync.dma_start(out=T1_sh[nrows - 1:nrows, 0:144], in_=s1[:])

            # ---- layer 1 edge phase (grouped super-gathers)
            for gi, (t0, t1) in enumerate(groups):
                tiles = list(range(t0, t1))
                SA = sum(Ka[t] for t in tiles)
                SB = sum(Kb[t] for t in tiles)
                GA = g1p.tile([128, max(SA, 1), T1_COLS], BF16, tag="GA")
                if SA:
                    o = ga_base[gi]
                    async_gather(GA[:, 0:SA, :], T1_sh[:],
                                 idxs[:, o // 16: o // 16 + SA * 8], 128 * SA, T1_COLS)
                GB = g1p.tile([128, max(SB, 1), T1_COLS], BF16, tag="GB")
                if SB:
                    o = gb_base[gi]
                    async_gather(GB[:, 0:SB, :], T1_sh[b_base:nrows, :],
                                 idxs[:, o // 16: o // 16 + SB * 8], 128 * SB, T1_COLS)
                for t in tiles:
                    ka, kb = Ka[t], Kb[t]
                    kt = ka + kb
                    aoff = (base_a[t] - ga_base[gi]) // 128
                    boff = (base_b[t] - gb_base[gi]) // 128
                    ps = ppa.tile([128, HEADS * (HID + 1)], F32, tag="agg")
                    rhs = rp.tile([128, kt + 1, HEADS * (HID + 1)], BF16, tag="rhs1")
                    rhs_v = rhs[:].rearrange("p k (h j) -> p k h j", h=HEADS)
                    for (G, goff, nk, ro) in ((GA, aoff, ka, 0), (GB, boff, kb, ka)):
                        if nk == 0:
                            continue
                        Gs = G[:, goff:goff + nk, :]
                        xl = smp.tile([128, nk, HEADS], BF16, tag="xl")
                        nc.vector.tensor_tensor(
                            out=xl[:], in0=Gs[:, :, 132:136],
                            in1=t1d_sb[:, None, t * 8 + 4:t * 8 + 8].to_broadcast([128, nk, HEADS]),
                            op=mybir.AluOpType.add)
                        e1 = smp.tile([128, nk, HEADS], BF16, tag="e1")
                        nc.scalar.activation(e1[:], xl[:], mybir.ActivationFunctionType.Exp)
                        e2 = smp.tile([128, nk, HEADS], BF16, tag="e2")
                        nc.scalar.activation(e2[:], xl[:], mybir.ActivationFunctionType.Exp, scale=NEG_SLOPE)
                        w = smp.tile([128, nk, HEADS], BF16, tag="w")
                        nc.vector.tensor_tensor(out=w[:], in0=e1[:], in1=e2[:], op=mybir.AluOpType.max)
                        nc.vector.tensor_tensor(
                            out=rhs_v[:, ro:ro + nk, :, :],
                            in0=Gs[:, :, 0:132].rearrange("p k (h j) -> p k h j", h=HEADS),
                            in1=w[:, :, :, None].to_broadcast([128, nk, HEADS, HID + 1]),
                            op=mybir.AluOpType.mult)
                    # self-loop chunk: w_self * [h|1]
                    xls = smp.tile([128, HEADS], BF16, tag="xls")
                    nc.vector.tensor_tensor(
                        out=xls[:], in0=t1d_sb[:, t * 8:t * 8 + 4],
                        in1=t1d_sb[:, t * 8 + 4:t * 8 + 8], op=mybir.AluOpType.add)
                    e1s = smp.tile([128, HEADS], BF16, tag="e1s")
                    nc.scalar.activation(e1s[:], xls[:], mybir.ActivationFunctionType.Exp)
                    e2s = smp.tile([128, HEADS], BF16, tag="e2s")
                    nc.scalar.activation(e2s[:], xls[:], mybir.ActivationFunctionType.Exp, scale=NEG_SLOPE)
                    ws = smp.tile([128, HEADS], BF16, tag="ws")
                    nc.vector.tensor_tensor(out=ws[:], in0=e1s[:], in1=e2s[:], op=mybir.AluOpType.max)
                    nc.vector.tensor_tensor(
                        out=rhs_v[:, kt, :, :],
                        in0=hball[:, t * 132:(t + 1) * 132].rearrange("p (h j) -> p h j", h=HEADS),
                        in1=ws[:, :, None].to_broadcast([128, HEADS, HID + 1]),
                        op=mybir.AluOpType.mult)
                    for cch in range(kt + 1):
                        nc.tensor.matmul(out=ps[:], lhsT=ident[:], rhs=rhs[:, cch, :],
                                         start=(cch == 0), stop=(cch == kt))
                    # epilogue: divide, +b1, ELU
                    ps_v = ps[:].rearrange("p (h j) -> p h j", h=HEADS)
                    rec = smp.tile([128, HEADS], F32, tag="rec")
                    nc.vector.reciprocal(out=rec[:], in_=ps_v[:, :, HID])
                    y1 = ppy.tile([128, 128], F32, tag="y")
                    nc.vector.tensor_tensor(
                        out=y1[:].rearrange("p (h j) -> p h j", h=HEADS),
                        in0=ps_v[:, :, 0:HID],
                        in1=rec[:, :, None].to_broadcast([128, HEADS, HID]),
                        op=mybir.AluOpType.mult)
                    nc.vector.tensor_tensor(
                        out=y1[:].rearrange("p (h j) -> p h j", h=HEADS),
                        in0=y1[:].rearrange("p (h j) -> p h j", h=HEADS),
                        in1=b1r[:].rearrange("p (h j) -> p h j", h=HEADS)[:, :, 0:HID],
                        op=mybir.AluOpType.add)
                    m1 = ep.tile([128, 128], F32, tag="m1")
                    nc.vector.tensor_scalar(out=m1[:], in0=y1[:], scalar1=0.0, scalar2=None,
                                            op0=mybir.AluOpType.min)
                    eE = ep.tile([128, 128], F32, tag="eE")
                    nc.scalar.activation(eE[:], m1[:], mybir.ActivationFunctionType.Exp)
                    r1 = ep.tile([128, 128], F32, tag="r1")
                    nc.vector.tensor_scalar(out=r1[:], in0=y1[:], scalar1=0.0, scalar2=-1.0,
                                            op0=mybir.AluOpType.max, op1=mybir.AluOpType.add)
                    h2 = ep.tile([128, 128], BF16, tag="h2")
                    nc.vector.tensor_tensor(out=h2[:], in0=eE[:], in1=r1[:], op=mybir.AluOpType.add)
                    # transpose h2, z = h2 @ W2big
                    pt = ppm.tile([128, 128], BF16, tag="misc")
                    nc.tensor.transpose(out=pt[:], in_=h2[:], identity=ident[:])
                    h2T = ep.tile([128, 128], BF16, tag="h2T")
                    nc.vector.tensor_copy(out=h2T[:], in_=pt[:])
                    psz = ppm.tile([128, HID + 2], F32, tag="misc")
                    nc.tensor.matmul(out=psz[:], lhsT=h2T[:], rhs=W2big[:], start=True, stop=True)
                    t2b = sp.tile([128, T2_COLS], BF16, tag="t2b")
                    nc.vector.tensor_tensor(out=t2b[:, 0:HID + 2], in0=psz[:], in1=b2r[:],
                                            op=mybir.AluOpType.add)
                    nc.vector.memset(t2b[:, HID + 2:HID + 3], 1.0)
                    nc.sync.dma_start(out=T2_own[t * 128:(t + 1) * 128, :], in_=t2b[:])
                    nc.vector.tensor_copy(out=zball[:, t * T2_USED:(t + 1) * T2_USED],
                                          in_=t2b[:, 0:T2_USED])
                    nc.vector.tensor_copy(out=t2d_sb[:, t * 2:(t + 1) * 2], in_=t2b[:, 32:34])

            # ---- allgather T2 + sentinel pokes
            nc.gpsimd.collective_compute(
                "AllGather", mybir.AluOpType.bypass, replica_groups=rgroups,
                ins=[T2_own[:]], outs=[T2_sh[1:1 + N_CORES * per_core, :]],
            )
            s2 = cp.tile([1, 36], BF16)
            nc.sync.dma_start(out=s2[:], in_=sent2_d[:])
            nc.sync.dma_start(out=T2_sh[0:1, 0:36], in_=s2[:])
            nc.sync.dma_start(out=T2_sh[nrows - 1:nrows, 0:36], in_=s2[:])

            # ---- layer 2 edge phase + output (grouped super-gathers)
            for gi, (t0, t1) in enumerate(groups):
                tiles2 = list(range(t0, t1))
                SA = sum(Ka[t] for t in tiles2)
                SB = sum(Kb[t] for t in tiles2)
                GA2 = g2p.tile([128, max(SA, 1), T2_COLS], BF16, tag="GA2")
                if SA:
                    o = ga_base[gi]
                    async_gather(GA2[:, 0:SA, :], T2_sh[:],
                                 idxs[:, o // 16: o // 16 + SA * 8], 128 * SA, T2_COLS)
                GB2 = g2p.tile([128, max(SB, 1), T2_COLS], BF16, tag="GB2")
                if SB:
                    o = gb_base[gi]
                    async_gather(GB2[:, 0:SB, :], T2_sh[b_base:nrows, :],
                                 idxs[:, o // 16: o // 16 + SB * 8], 128 * SB, T2_COLS)
                for t in tiles2:
                    ka, kb = Ka[t], Kb[t]
                    kt = ka + kb
                    aoff = (base_a[t] - ga_base[gi]) // 128
                    boff = (base_b[t] - gb_base[gi]) // 128
                    ps2 = ppa.tile([128, T2_USED], F32, tag="agg")
                    rhs2 = rp.tile([128, kt + 1, T2_USED], BF16, tag="rhs2")
                    for (G2, goff, nk, ro) in ((GA2, aoff, ka, 0), (GB2, boff, kb, ka)):
                        if nk == 0:
                            continue
                        Gs = G2[:, goff:goff + nk, :]
                        xl2 = smp.tile([128, nk, 1], BF16, tag="xl2")
                        nc.vector.tensor_tensor(
                            out=xl2[:], in0=Gs[:, :, 32:33],
                            in1=t2d_sb[:, None, t * 2 + 1:t * 2 + 2].to_broadcast([128, nk, 1]),
                            op=mybir.AluOpType.add)
                        e1b = smp.tile([128, nk, 1], BF16, tag="e1b")
                        nc.scalar.activation(e1b[:], xl2[:], mybir.ActivationFunctionType.Exp)
                        e2b = smp.tile([128, nk, 1], BF16, tag="e2b")
                        nc.scalar.activation(e2b[:], xl2[:], mybir.ActivationFunctionType.Exp, scale=NEG_SLOPE)
                        w2 = smp.tile([128, nk, 1], BF16, tag="w2")
                        nc.vector.tensor_tensor(out=w2[:], in0=e1b[:], in1=e2b[:], op=mybir.AluOpType.max)
                        nc.vector.tensor_tensor(
                            out=rhs2[:, ro:ro + nk, :],
                            in0=Gs[:, :, 0:T2_USED],
                            in1=w2[:, :, :].to_broadcast([128, nk, T2_USED]),
                            op=mybir.AluOpType.mult)
                    # self-loop chunk
                    xl2s = smp.tile([128, 1], BF16, tag="xl2s")
                    nc.vector.tensor_tensor(
                        out=xl2s[:], in0=t2d_sb[:, t * 2:t * 2 + 1],
                        in1=t2d_sb[:, t * 2 + 1:t * 2 + 2], op=mybir.AluOpType.add)
                    e1bs = smp.tile([128, 1], BF16, tag="e1bs")
                    nc.scalar.activation(e1bs[:], xl2s[:], mybir.ActivationFunctionType.Exp)
                    e2bs = smp.tile([128, 1], BF16, tag="e2bs")
                    nc.scalar.activation(e2bs[:], xl2s[:], mybir.ActivationFunctionType.Exp, scale=NEG_SLOPE)
                    w2s = smp.tile([128, 1], BF16, tag="w2s")
                    nc.vector.tensor_tensor(out=w2s[:], in0=e1bs[:], in1=e2bs[:], op=mybir.AluOpType.max)
                    nc.vector.tensor_tensor(
                        out=rhs2[:, kt, :],
                        in0=zball[:, t * T2_USED:(t + 1) * T2_USED],
                        in1=w2s[:].to_broadcast([128, T2_USED]),
                        op=mybir.AluOpType.mult)
                    for cch in range(kt + 1):
                        nc.tensor.matmul(out=ps2[:], lhsT=ident[:], rhs=rhs2[:, cch, :],
                                         start=(cch == 0), stop=(cch == kt))
                    rec2 = smp.tile([128, 1], F32, tag="rec2")
                    nc.vector.reciprocal(out=rec2[:], in_=ps2[:, HID + 2:HID + 3])
                    y2 = ppy.tile([128, HID], F32, tag="y")
                    nc.vector.tensor_tensor(
                        out=y2[:], in0=ps2[:, 0:HID],
                        in1=rec2[:].to_broadcast([128, HID]),
                        op=mybir.AluOpType.mult)
                    m2 = ep.tile([128, HID], F32, tag="m2")
                    nc.vector.tensor_scalar(out=m2[:], in0=y2[:], scalar1=0.0, scalar2=None,
                                            op0=mybir.AluOpType.min)
                    eE2 = ep.tile([128, HID], F32, tag="eE2")
                    nc.scalar.activation(eE2[:], m2[:], mybir.ActivationFunctionType.Exp)
                    r2 = ep.tile([128, HID], F32, tag="r2")
                    nc.vector.tensor_scalar(out=r2[:], in0=y2[:], scalar1=0.0, scalar2=-1.0,
                                            op0=mybir.AluOpType.max, op1=mybir.AluOpType.add)
                    h3 = ep.tile([128, HID], BF16, tag="h3")
                    nc.vector.tensor_tensor(out=h3[:], in0=eE2[:], in1=r2[:], op=mybir.AluOpType.add)
                    pt2 = ppm.tile([128, 128], BF16, tag="misc")
                    nc.tensor.transpose(out=pt2[:HID, :], in_=h3[:], identity=ident[:])
                    h3T = ep.tile([HID, 128], BF16, tag="h3T")
                    nc.vector.tensor_copy(out=h3T[:], in_=pt2[:HID, :])
                    psf = ppm.tile([128, OUT_CH], F32, tag="misc")
                    nc.tensor.matmul(out=psf[:], lhsT=h3T[:], rhs=Wout[:], start=True, stop=True)
                    outf = ep.tile([128, OUT_CH], F32, tag="outf")
                    nc.vector.tensor_tensor(out=outf[:], in0=psf[:], in1=boutr[:],
                                            op=mybir.AluOpType.add)
                    nc.sync.dma_start(out=out_d[t * 128:(t + 1) * 128, :], in_=outf[:])

    nc.compile()
    return nc


def _run(inputs, trace=False):
    meta, in_maps = _prep(**inputs)
    nc = _build(meta)
    res = run_bass_kernel_spmd(nc, in_maps, core_ids=list(range(N_CORES)), trace=trace)
    outg = np.concatenate([res.results[c]["out"] for c in range(N_CORES)], axis=0)
    out_nodes = np.empty((meta["n_pad"], OUT_CH), np.float32)
    out_nodes[meta["perm_rows"]] = outg
    return out_nodes[:meta["N"]], res


def kernel(**inputs):
    out, _ = _run(inputs, trace=False)
    return out


# revision 18
# speedup vs baseline: 1.9999x; 1.0468x over previous
"""GAT (2-layer, 4-head then 1-head) on 8 Trainium2 NeuronCores.

Strategy
--------
- Nodes are permuted: globally degree-sorted, dealt round-robin to 8 cores
  (edge balance + nearly-identical degree profiles per core), then each
  core's nodes are laid out in 128-node dst tiles. Tiles are degree-uniform,
  so per-dst edge lists pad to the tile max with small waste.
- Edges land in a "slot grid" [128 dst x K slots] per tile: slot-chunk c is
  128 edges whose partition IS the dst row. The aggregation matmul then has
  an identity stationary operand.
- Per-edge messages are fetched with dma_gather (int16 indices) issued as
  prepare_only descriptors + trigger_dma so transfers run async on 4 SWDGE
  queues. The bottleneck is Q7 descriptor generation (~8ns/idx), so the
  index count is minimized:
    * self-loops are NOT gathered: each tile's self contribution is built
      locally as one extra rhs chunk (w_self * [h|1]).
    * the int16 range split uses OVERLAPPING views (A = rows 0..32767 from
      base 0, B = rows 8194..40961 from base 8194). Edges with src row in
      [8194, 32767] can go to either side and are assigned to balance the
      per-tile (Ka, Kb) caps, nearly halving the padding.
- Sentinel rows (0 and last) have attention logits of -60000 so padded
  slots get weight exp(-inf)=0 and contribute nothing.
- Softmax without max-subtraction: w_e = exp(leakyrelu(x)) = max(exp(x),
  exp(0.2 x)), accumulated per dst in PSUM along with the denominator
  (ones-column trick), divided once per node.
- Layer outputs are transformed (W2 / W_out) per tile; node tables are
  AllGather'd across cores between layers.
"""

import numpy as np

import concourse.bacc as bacc
import concourse.mybir as mybir
import concourse.tile as tile
from concourse.bass_utils import run_bass_kernel_spmd

F32 = mybir.dt.float32
BF16 = mybir.dt.float16  # NB: fp16 (renamed var kept)
I16 = mybir.dt.int16

IN_CH = 128
HID = 32
HEADS = 4
OUT_CH = 112
NEG_SLOPE = 0.2
NEG_BIG = -60000.0

# Layer-1 table row: [h(128) | al_src(4) | al_dst(4) | one(1) | pad] = 256 bf16 (512B)
T1_COLS = 256
T1_OWN = T1_COLS
# Layer-2 table row: [z(32) | as2(1) | ad2(1) | one(1) | pad] = 128 bf16 (256B)
T2_COLS = 128
T2_USED = 35
T2_OWN = T2_COLS

N_CORES = 8
GCAP = 56
N_AG_CHUNKS = 4


def _prep(x, edge_index, W1, a_src1, a_dst1, b1, W2, a_src2, a_dst2, b2, W_out, b_out):
    """Host-side graph preprocessing. Returns (meta, per-core inputs)."""
    N = x.shape[0]
    per_core = -(-N // (N_CORES * 128)) * 128
    n_pad = per_core * N_CORES
    NT = per_core // 128
    nrows = n_pad + 2  # + 2 sentinel rows
    BB = nrows - 1 - 32767  # first row of the B view; B covers rows BB..nrows-1
    assert BB >= 0 and BB <= 32767

    src = np.asarray(edge_index[0], np.int64)
    dst = np.asarray(edge_index[1], np.int64)
    E2 = src.shape[0]

    deg_s = np.bincount(dst, minlength=n_pad) + 1  # incl self loop, for sorting
    order = np.argsort(deg_s, kind="stable")  # ascending degree, pads first
    rank = np.empty(n_pad, np.int64)
    rank[order] = np.arange(n_pad)
    core_of = rank % N_CORES
    pos_in_core = rank // N_CORES
    grow = core_of * per_core + pos_in_core      # local row (core-major, for dst/output)
    CHK = per_core // N_AG_CHUNKS
    chunk = pos_in_core // CHK
    pic = pos_in_core % CHK
    # table row order is chunk-major so each AllGather chunk lands contiguously
    trow = 1 + chunk * (N_CORES * CHK) + core_of * CHK + pic
    perm_rows = np.empty(n_pad, np.int64)
    perm_rows[grow] = np.arange(n_pad)

    sr = trow[src]
    dr = grow[dst]
    # class: 0 = A-only (row < BB), 1 = flex, 2 = B-only (row > 32767)
    cls = np.where(sr < BB, 0, np.where(sr > 32767, 2, 1)).astype(np.int64)

    deg = np.bincount(dr, minlength=n_pad)          # per dst row, no self
    nA = np.bincount(dr[cls == 0], minlength=n_pad)
    nB = np.bincount(dr[cls == 2], minlength=n_pad)
    tile_of = (np.arange(n_pad) % per_core) // 128

    Ka = np.zeros(NT, np.int64)
    Kb = np.zeros(NT, np.int64)
    for t in range(NT):
        sel = tile_of == t
        ka = nA[sel].max()
        kb = nB[sel].max()
        ka += max(0, deg[sel].max() - ka - kb)  # ensure Ka+Kb >= maxdeg
        Ka[t], Kb[t] = ka, kb
    Kt = Ka + Kb

    # per-dst A-count: a = max(nA, deg - Kb[tile])
    a_of = np.maximum(nA, deg - Kb[tile_of])

    # sort edges by (dst row, class); first a_of[d] edges of each run -> A
    eorder = np.lexsort((cls, dr))
    dr_s = dr[eorder]
    sr_s = sr[eorder]
    newrun = np.empty(E2, bool)
    newrun[0] = True
    newrun[1:] = dr_s[1:] != dr_s[:-1]
    run_start_idx = np.flatnonzero(newrun)
    run_id = np.cumsum(newrun) - 1
    pos_in_run = np.arange(E2) - run_start_idx[run_id]
    in_A = pos_in_run < a_of[dr_s]
    # slot within its side
    slot = np.where(in_A, pos_in_run, pos_in_run - a_of[dr_s])

    # adaptive groups: consecutive tiles, sum of slots <= GCAP
    groups = []
    t = 0
    while t < NT:
        e = t
        tot = 0
        while e < NT and (e == t or tot + Kt[e] <= GCAP):
            tot += Kt[e]
            e += 1
        groups.append((t, e))
        t = e
    base_a = np.zeros(NT, np.int64)
    base_b = np.zeros(NT, np.int64)
    ga_base = []
    gb_base = []
    off = 0
    for (t0, t1) in groups:
        ga_base.append(off)
        for t in range(t0, t1):
            base_a[t] = off
            off += 128 * Ka[t]
        gb_base.append(off)
        for t in range(t0, t1):
            base_b[t] = off
            off += 128 * Kb[t]
    totidx = off

    # default stream = sentinels (A: row 0; B: local 32767 = last row)
    default = np.zeros(totidx, np.int16)
    for t in range(NT):
        default[base_a[t]:base_a[t] + 128 * Ka[t]] = 0
        default[base_b[t]:base_b[t] + 128 * Kb[t]] = 32767
    streams = np.tile(default, (N_CORES, 1))

    e_core = dr_s // per_core
    loc = dr_s % per_core
    tl = loc // 128
    p = loc % 128
    posA = base_a[tl] + slot * 128 + p
    posB = base_b[tl] + slot * 128 + p
    pos = np.where(in_A, posA, posB)
    val = np.where(in_A, sr_s, sr_s - BB).astype(np.int16)
    assert val.min() >= 0
    streams[e_core, pos] = val

    # wrap for dma_gather: wrapped[p, j] = flat[j*16 + p%16]
    assert totidx % 16 == 0
    idx_wrapped = np.empty((N_CORES, 128, totidx // 16), np.int16)
    for c in range(N_CORES):
        w16 = streams[c].reshape(-1, 16).T  # [16, totidx/16]
        idx_wrapped[c] = np.tile(w16, (8, 1))

    # x slices (table-row order per core), fp16 for fast PE loads
    xp = np.zeros((n_pad, IN_CH), np.float32)
    xp[:N] = np.asarray(x, np.float32)
    x_slices = np.empty((N_CORES, IN_CH, per_core), np.float16)
    for c in range(N_CORES):
        nodes = perm_rows[c * per_core:(c + 1) * per_core]
        x_slices[c] = xp[nodes].T.astype(np.float16)

    # weight packs
    W1 = np.asarray(W1, np.float32)
    Bsrc = np.zeros((HEADS * HID, HEADS), np.float32)
    Bdst = np.zeros((HEADS * HID, HEADS), np.float32)
    for h in range(HEADS):
        Bsrc[h * HID:(h + 1) * HID, h] = np.asarray(a_src1[h], np.float32)
        Bdst[h * HID:(h + 1) * HID, h] = np.asarray(a_dst1[h], np.float32)
    W1cols = []
    for h in range(HEADS):
        W1cols.append(W1[:, h * HID:(h + 1) * HID])
        W1cols.append(np.zeros((IN_CH, 1), np.float32))  # ones-slot
    W1big = np.concatenate(W1cols + [W1 @ Bsrc, W1 @ Bdst], axis=1)  # [128, 140]
    W2 = np.asarray(W2, np.float32)
    W2big = np.concatenate(
        [W2, W2 @ np.asarray(a_src2, np.float32).T, W2 @ np.asarray(a_dst2, np.float32).T],
        axis=1,
    )  # [128, 34]
    b1v = np.asarray(b1, np.float32).reshape(HEADS, HID)
    b1i = np.zeros((HEADS, HID + 1), np.float32)
    b1i[:, :HID] = b1v
    b1_rep = np.tile(b1i.reshape(1, -1), (128, 1))                            # [128,132]
    b2_rep = np.zeros((128, HID + 2), np.float32)
    b2_rep[:, :HID] = np.asarray(b2, np.float32)[None, :]
    bout_rep = np.tile(np.asarray(b_out, np.float32)[None, :], (128, 1))     # [128,112]
    ident = np.eye(128, dtype=np.float32)

    bf16 = np.float16
    sent1 = np.zeros((1, 144), bf16)
    sent1[0, 132:140] = NEG_BIG
    sent2 = np.zeros((1, 36), bf16)
    sent2[0, 32:34] = NEG_BIG

    meta = dict(
        N=N, E2=E2, n_pad=n_pad, per_core=per_core, NT=NT, nrows=nrows,
        b_base=BB, Ka=Ka.tolist(), Kb=Kb.tolist(),
        base_a=base_a.tolist(), base_b=base_b.tolist(), totidx=totidx,
        ga_base=ga_base, gb_base=gb_base, groups=groups,
        perm_rows=perm_rows,
    )
    shared = dict(
        W1big=W1big.astype(bf16), W2big=W2big.astype(bf16),
        Wout=np.asarray(W_out, np.float32).astype(bf16),
        b1_rep=b1_rep, b2_rep=b2_rep, bout_rep=bout_rep, ident=ident.astype(bf16),
        sent1=sent1, sent2=sent2,
    )
    in_maps = []
    for c in range(N_CORES):
        m = dict(shared)
        m["x_slice"] = np.ascontiguousarray(x_slices[c])
        m["idx_flat"] = np.ascontiguousarray(idx_wrapped[c])
        in_maps.append(m)
    return meta, in_maps


def _build(meta):
    per_core, NT, nrows, b_base = meta["per_core"], meta["NT"], meta["nrows"], meta["b_base"]
    Ka, Kb = meta["Ka"], meta["Kb"]
    base_a, base_b, totidx = meta["base_a"], meta["base_b"], meta["totidx"]
    ga_base, gb_base, groups = meta["ga_base"], meta["gb_base"], meta["groups"]

    nc = bacc.Bacc("TRN2", num_devices=N_CORES, num_swdge_queues=4,
                   dynamic_dma_scratch_size=65536)

    x_slice = nc.dram_tensor("x_slice", [IN_CH, per_core], BF16, kind="ExternalInput")
    idx_flat = nc.dram_tensor("idx_flat", [128, totidx // 16], I16, kind="ExternalInput")
    W1big_d = nc.dram_tensor("W1big", [128, 140], BF16, kind="ExternalInput")
    W2big_d = nc.dram_tensor("W2big", [128, HID + 2], BF16, kind="ExternalInput")
    Wout_d = nc.dram_tensor("Wout", [HID, OUT_CH], BF16, kind="ExternalInput")
    b1_d = nc.dram_tensor("b1_rep", [128, 132], F32, kind="ExternalInput")
    b2_d = nc.dram_tensor("b2_rep", [128, HID + 2], F32, kind="ExternalInput")
    bout_d = nc.dram_tensor("bout_rep", [128, OUT_CH], F32, kind="ExternalInput")
    ident_d = nc.dram_tensor("ident", [128, 128], BF16, kind="ExternalInput")
    sent1_d = nc.dram_tensor("sent1", [1, 144], BF16, kind="ExternalInput")
    sent2_d = nc.dram_tensor("sent2", [1, 36], BF16, kind="ExternalInput")

    CHK = per_core // N_AG_CHUNKS
    TPC = NT // N_AG_CHUNKS  # tiles per AG chunk
    T1_ownc = [nc.dram_tensor(f"T1_own{j}", [CHK, T1_OWN], BF16, kind="Internal")
               for j in range(N_AG_CHUNKS)]
    T1_sh = nc.dram_tensor("T1_sh", [nrows, T1_COLS], BF16, kind="Internal", addr_space="Shared")
    T2_ownc = [nc.dram_tensor(f"T2_own{j}", [CHK, T2_OWN], BF16, kind="Internal")
               for j in range(N_AG_CHUNKS)]
    T2_sh = nc.dram_tensor("T2_sh", [nrows, T2_COLS], BF16, kind="Internal", addr_space="Shared")
    out_d = nc.dram_tensor("out", [per_core, OUT_CH], F32, kind="ExternalOutput")

    rgroups = [list(range(N_CORES))]
    qctr = [0]
    qsems = [nc.alloc_semaphore(f"gsem{q}") for q in range(4)]

    def qn():
        q = qctr[0] % 4
        qctr[0] += 1
        return q

    import os
    ASYNC = os.environ.get("GAT_ASYNC_GATHER", "0") == "1"

    def async_gather(out_ap, table_ap, idx_ap, nidx, cols):
        """Prep descriptors on a rotating SWDGE queue; fire the DMA async."""
        q = qn()
        if ASYNC:
            nc.gpsimd.dma_gather(
                out_ap, table_ap, idx_ap, nidx, nidx, cols,
                prepare_only=True, sem=qsems[q], queue_num=q, single_packet=False)
            nc.gpsimd.trigger_dma(count=None, queue_num=q)
        else:
            nc.gpsimd.dma_gather(
                out_ap, table_ap, idx_ap, nidx, nidx, cols,
                queue_num=q, single_packet=False)

    with tile.TileContext(nc) as tc:
        with (
            tc.tile_pool(name="const", bufs=1) as cp,
            tc.tile_pool(name="xa", bufs=2) as xap,
            tc.tile_pool(name="stage", bufs=3) as sp,
            tc.tile_pool(name="g1", bufs=2) as g1p,
            tc.tile_pool(name="g2", bufs=2) as g2p,
            tc.tile_pool(name="small", bufs=4) as smp,
            tc.tile_pool(name="rhs", bufs=2) as rp,
            tc.tile_pool(name="epi", bufs=3) as ep,
            tc.tile_pool(name="psa", bufs=3, space="PSUM") as ppa,
            tc.tile_pool(name="psm", bufs=3, space="PSUM") as ppm,
            tc.tile_pool(name="psy", bufs=2, space="PSUM") as ppy,
        ):
            # ---- consts to SBUF
            W1big = cp.tile([128, 140], BF16)
            nc.sync.dma_start(out=W1big[:], in_=W1big_d[:])
            W2big = cp.tile([128, HID + 2], BF16)
            nc.sync.dma_start(out=W2big[:], in_=W2big_d[:])
            Wout = cp.tile([HID, OUT_CH], BF16)
            nc.sync.dma_start(out=Wout[:], in_=Wout_d[:])
            b1r = cp.tile([128, 132], F32)
            nc.sync.dma_start(out=b1r[:], in_=b1_d[:])
            b2r = cp.tile([128, HID + 2], F32)
            nc.sync.dma_start(out=b2r[:], in_=b2_d[:])
            boutr = cp.tile([128, OUT_CH], F32)
            nc.sync.dma_start(out=boutr[:], in_=bout_d[:])
            ident = cp.tile([128, 128], BF16)
            nc.sync.dma_start(out=ident[:], in_=ident_d[:])
            idxs = cp.tile([128, totidx // 16], I16)
            nc.sync.dma_start(out=idxs[:], in_=idx_flat[:])
            # SBUF-resident own-node data (no DRAM roundtrip)
            t1d_sb = cp.tile([128, NT * 8], BF16)    # [al_src(4)|al_dst(4)] per tile
            t2d_sb = cp.tile([128, NT * 2], BF16)    # [as2|ad2] per tile
            hball = cp.tile([128, NT * 132], BF16)   # own [h|1]x4 rows per tile
            zball = cp.tile([128, NT * T2_USED], BF16)  # own [z|as2|ad2|1] per tile

            # ---- phase A: own node tiles -> T1_own
            for t in range(NT):
                xa = xap.tile([128, 128], BF16)
                nc.sync.dma_start(out=xa[:], in_=x_slice[:, t * 128:(t + 1) * 128])
                ps = ppa.tile([128, 140], F32, tag="agg")
                nc.tensor.matmul(out=ps[:], lhsT=xa[:], rhs=W1big[:], start=True, stop=True)
                hb = sp.tile([128, T1_OWN], BF16)
                nc.vector.tensor_copy(out=hb[:, 0:140], in_=ps[:])
                hb_v = hb[:, 0:132].rearrange("p (h j) -> p h j", h=HEADS)
                nc.vector.memset(hb_v[:, :, HID:HID + 1], 1.0)
                lt = (t % TPC) * 128
                nc.sync.dma_start(out=T1_ownc[t // TPC][lt:lt + 128, :], in_=hb[:])
                nc.scalar.activation(t1d_sb[:, t * 8:(t + 1) * 8], ps[:, 132:140],
                                     mybir.ActivationFunctionType.Copy)
                nc.scalar.activation(hball[:, t * 132:(t + 1) * 132], hb[:, 0:132],
                                     mybir.ActivationFunctionType.Copy)

            # ---- allgather T1 (chunked; chunk j fires when its tiles are stored)
            for j in range(N_AG_CHUNKS):
                o = 1 + j * CHK * N_CORES
                nc.gpsimd.collective_compute(
                    "AllGather", mybir.AluOpType.bypass, replica_groups=rgroups,
                    ins=[T1_ownc[j][:]], outs=[T1_sh[o:o + CHK * N_CORES, :]],
                )
            s1 = cp.tile([1, 144], BF16)
            nc.sync.dma_start(out=s1[:], in_=sent1_d[:])
            nc.sync.dma_start(out=T1_sh[0:1, 0:144], in_=s1[:])
            nc.sync.dma_start(out=T1_sh[nrows - 1:nrows, 0:144], in_=s1[:])

            # ---- layer 1 edge phase (grouped super-gathers)
            for gi, (t0, t1) in enumerate(groups):
                tiles = list(range(t0, t1))
                SA = sum(Ka[t] for t in tiles)
                SB = sum(Kb[t] for t in tiles)
                GA = g1p.tile([128, max(SA, 1), T1_COLS], BF16, tag="GA")
                if SA:
                    o = ga_base[gi]
                    async_gather(GA[:, 0:SA, :], T1_sh[:],
                                 idxs[:, o // 16: o // 16 + SA * 8], 128 * SA, T1_COLS)
                GB = g1p.tile([128, max(SB, 1), T1_COLS], BF16, tag="GB")
                if SB:
                    o = gb_base[gi]
                    async_gather(GB[:, 0:SB, :], T1_sh[b_base:nrows, :],
                                 idxs[:, o // 16: o // 16 + SB * 8], 128 * SB, T1_COLS)
                for t in tiles:
                    ka, kb = Ka[t], Kb[t]
                    kt = ka + kb
                    aoff = (base_a[t] - ga_base[gi]) // 128
                    boff = (base_b[t] - gb_base[gi]) // 128
                    ps = ppa.tile([128, HEADS * (HID + 1)], F32, tag="agg")
                    rhs = rp.tile([128, kt + 1, HEADS * (HID + 1)], BF16, tag="rhs1")
                    rhs_v = rhs[:].rearrange("p k (h j) -> p k h j", h=HEADS)
                    for (G, goff, nk, ro) in ((GA, aoff, ka, 0), (GB, boff, kb, ka)):
                        if nk == 0:
                            continue
                        Gs = G[:, goff:goff + nk, :]
                        xl = smp.tile([128, nk, HEADS], BF16, tag="xl")
                        nc.vector.tensor_tensor(
                            out=xl[:], in0=Gs[:, :, 132:136],
                            in1=t1d_sb[:, None, t * 8 + 4:t * 8 + 8].to_broadcast([128, nk, HEADS]),
                            op=mybir.AluOpType.add)
                        e1 = smp.tile([128, nk, HEADS], BF16, tag="e1")
                        nc.scalar.activation(e1[:], xl[:], mybir.ActivationFunctionType.Exp)
                        e2 = smp.tile([128, nk, HEADS], BF16, tag="e2")
                        nc.scalar.activation(e2[:], xl[:], mybir.ActivationFunctionType.Exp, scale=NEG_SLOPE)
                        w = smp.tile([128, nk, HEADS], BF16, tag="w")
                        nc.vector.tensor_tensor(out=w[:], in0=e1[:], in1=e2[:], op=mybir.AluOpType.max)
                        nc.vector.tensor_tensor(
                            out=rhs_v[:, ro:ro + nk, :, :],
                            in0=Gs[:, :, 0:132].rearrange("p k (h j) -> p k h j", h=HEADS),
                            in1=w[:, :, :, None].to_broadcast([128, nk, HEADS, HID + 1]),
                            op=mybir.AluOpType.mult)
                    # self-loop chunk: w_self * [h|1]
                    xls = smp.tile([128, HEADS], BF16, tag="xls")
                    nc.vector.tensor_tensor(
                        out=xls[:], in0=t1d_sb[:, t * 8:t * 8 + 4],
                        in1=t1d_sb[:, t * 8 + 4:t * 8 + 8], op=mybir.AluOpType.add)
                    e1s = smp.tile([128, HEADS], BF16, tag="e1s")
                    nc.scalar.activation(e1s[:], xls[:], mybir.ActivationFunctionType.Exp)
                    e2s = smp.tile([128, HEADS], BF16, tag="e2s")
                    nc.scalar.activation(e2s[:], xls[:], mybir.ActivationFunctionType.Exp, scale=NEG_SLOPE)
                    ws = smp.tile([128, HEADS], BF16, tag="ws")
                    nc.vector.tensor_tensor(out=ws[:], in0=e1s[:], in1=e2s[:], op=mybir.AluOpType.max)
                    nc.vector.tensor_tensor(
                        out=rhs_v[:, kt, :, :],
                        in0=hball[:, t * 132:(t + 1) * 132].rearrange("p (h j) -> p h j", h=HEADS),
                        in1=ws[:, :, None].to_broadcast([128, HEADS, HID + 1]),
                        op=mybir.AluOpType.mult)
                    for cch in range(kt + 1):
                        nc.tensor.matmul(out=ps[:], lhsT=ident[:], rhs=rhs[:, cch, :],
                                         start=(cch == 0), stop=(cch == kt))
                    # epilogue: divide, +b1, ELU
                    ps_v = ps[:].rearrange("p (h j) -> p h j", h=HEADS)
                    rec = smp.tile([128, HEADS], F32, tag="rec")
                    nc.vector.reciprocal(out=rec[:], in_=ps_v[:, :, HID])
                    y1 = ppy.tile([128, 128], F32, tag="y")
                    nc.vector.tensor_tensor(
                        out=y1[:].rearrange("p (h j) -> p h j", h=HEADS),
                        in0=ps_v[:, :, 0:HID],
                        in1=rec[:, :, None].to_broadcast([128, HEADS, HID]),
                        op=mybir.AluOpType.mult)
                    nc.vector.tensor_tensor(
                        out=y1[:].rearrange("p (h j) -> p h j", h=HEADS),
                        in0=y1[:].rearrange("p (h j) -> p h j", h=HEADS),
                        in1=b1r[:].rearrange("p (h j) -> p h j", h=HEADS)[:, :, 0:HID],
                        op=mybir.AluOpType.add)
                    m1 = ep.tile([128, 128], F32, tag="m1")
                    nc.vector.tensor_scalar(out=m1[:], in0=y1[:], scalar1=0.0, scalar2=None,
                                            op0=mybir.AluOpType.min)
                    eE = ep.tile([128, 128], F32, tag="eE")
                    nc.scalar.activation(eE[:], m1[:], mybir.ActivationFunctionType.Exp)
                    r1 = ep.tile([128, 128], F32, tag="r1")
                    nc.vector.tensor_scalar(out=r1[:], in0=y1[:], scalar1=0.0, scalar2=-1.0,
                                            op0=mybir.AluOpType.max, op1=mybir.AluOpType.add)
                    h2 = ep.tile([128, 128], BF16, tag="h2")
                    nc.vector.tensor_tensor(out=h2[:], in0=eE[:], in1=r1[:], op=mybir.AluOpType.add)
                    # transpose h2, z = h2 @ W2big
                    pt = ppm.tile([128, 128], BF16, tag="misc")
                    nc.tensor.transpose(out=pt[:], in_=h2[:], identity=ident[:])
                    h2T = ep.tile([128, 128], BF16, tag="h2T")
                    nc.scalar.activation(h2T[:], pt[:], mybir.ActivationFunctionType.Copy)
                    psz = ppm.tile([128, HID + 2], F32, tag="misc")
                    nc.tensor.matmul(out=psz[:], lhsT=h2T[:], rhs=W2big[:], start=True, stop=True)
                    t2b = sp.tile([128, T2_OWN], BF16, tag="t2b")
                    nc.vector.tensor_tensor(out=t2b[:, 0:HID + 2], in0=psz[:], in1=b2r[:],
                                            op=mybir.AluOpType.add)
                    nc.vector.memset(t2b[:, HID + 2:HID + 3], 1.0)
                    lt2 = (t % TPC) * 128
                    nc.sync.dma_start(out=T2_ownc[t // TPC][lt2:lt2 + 128, :], in_=t2b[:])
                    nc.scalar.activation(zball[:, t * T2_USED:(t + 1) * T2_USED],
                                         t2b[:, 0:T2_USED], mybir.ActivationFunctionType.Copy)
                    nc.scalar.activation(t2d_sb[:, t * 2:(t + 1) * 2], t2b[:, 32:34],
                                         mybir.ActivationFunctionType.Copy)

            # ---- allgather T2 (chunked) + sentinel pokes
            for j in range(N_AG_CHUNKS):
                o = 1 + j * CHK * N_CORES
                nc.gpsimd.collective_compute(
                    "AllGather", mybir.AluOpType.bypass, replica_groups=rgroups,
                    ins=[T2_ownc[j][:]], outs=[T2_sh[o:o + CHK * N_CORES, :]],
                )
            s2 = cp.tile([1, 36], BF16)
            nc.sync.dma_start(out=s2[:], in_=sent2_d[:])
            nc.sync.dma_start(out=T2_sh[0:1, 0:36], in_=s2[:])
            nc.sync.dma_start(out=T2_sh[nrows - 1:nrows, 0:36], in_=s2[:])

            # ---- layer 2 edge phase + output (grouped super-gathers)
            for gi, (t0, t1) in enumerate(groups):
                tiles2 = list(range(t0, t1))
                SA = sum(Ka[t] for t in tiles2)
                SB = sum(Kb[t] for t in tiles2)
                GA2 = g2p.tile([128, max(SA, 1), T2_COLS], BF16, tag="GA2")
                if SA:
                    o = ga_base[gi]
                    async_gather(GA2[:, 0:SA, :], T2_sh[:],
                                 idxs[:, o // 16: o // 16 + SA * 8], 128 * SA, T2_COLS)
                GB2 = g2p.tile([128, max(SB, 1), T2_COLS], BF16, tag="GB2")
                if SB:
                    o = gb_base[gi]
                    async_gather(GB2[:, 0:SB, :], T2_sh[b_base:nrows, :],
                                 idxs[:, o // 16: o // 16 + SB * 8], 128 * SB, T2_COLS)
                for t in tiles2:
                    ka, kb = Ka[t], Kb[t]
                    kt = ka + kb
                    aoff = (base_a[t] - ga_base[gi]) // 128
                    boff = (base_b[t] - gb_base[gi]) // 128
                    ps2 = ppa.tile([128, T2_USED], F32, tag="agg")
                    rhs2 = rp.tile([128, kt + 1, T2_USED], BF16, tag="rhs2")
                    for (G2, goff, nk, ro) in ((GA2, aoff, ka, 0), (GB2, boff, kb, ka)):
                        if nk == 0:
                            continue
                        Gs = G2[:, goff:goff + nk, :]
                        xl2 = smp.tile([128, nk, 1], BF16, tag="xl2")
                        nc.vector.tensor_tensor(
                            out=xl2[:], in0=Gs[:, :, 32:33],
                            in1=t2d_sb[:, None, t * 2 + 1:t * 2 + 2].to_broadcast([128, nk, 1]),
                            op=mybir.AluOpType.add)
                        e1b = smp.tile([128, nk, 1], BF16, tag="e1b")
                        nc.scalar.activation(e1b[:], xl2[:], mybir.ActivationFunctionType.Exp)
                        e2b = smp.tile([128, nk, 1], BF16, tag="e2b")
                        nc.scalar.activation(e2b[:], xl2[:], mybir.ActivationFunctionType.Exp, scale=NEG_SLOPE)
                        w2 = smp.tile([128, nk, 1], BF16, tag="w2")
                        nc.vector.tensor_tensor(out=w2[:], in0=e1b[:], in1=e2b[:], op=mybir.AluOpType.max)
                        nc.vector.tensor_tensor(
                            out=rhs2[:, ro:ro + nk, :],
                            in0=Gs[:, :, 0:T2_USED],
                            in1=w2[:, :, :].to_broadcast([128, nk, T2_USED]),
                            op=mybir.AluOpType.mult)
                    # self-loop chunk
                    xl2s = smp.tile([128, 1], BF16, tag="xl2s")
                    nc.vector.tensor_tensor(
                        out=xl2s[:], in0=t2d_sb[:, t * 2:t * 2 + 1],
                        in1=t2d_sb[:, t * 2 + 1:t * 2 + 2], op=mybir.AluOpType.add)
                    e1bs = smp.tile([128, 1], BF16, tag="e1bs")
                    nc.scalar.activation(e1bs[:], xl2s[:], mybir.ActivationFunctionType.Exp)
                    e2bs = smp.tile([128, 1], BF16, tag="e2bs")
                    nc.scalar.activation(e2bs[:], xl2s[:], mybir.ActivationFunctionType.Exp, scale=NEG_SLOPE)
                    w2s = smp.tile([128, 1], BF16, tag="w2s")
                    nc.vector.tensor_tensor(out=w2s[:], in0=e1bs[:], in1=e2bs[:], op=mybir.AluOpType.max)
                    nc.vector.tensor_tensor(
                        out=rhs2[:, kt, :],
                        in0=zball[:, t * T2_USED:(t + 1) * T2_USED],
                        in1=w2s[:].to_broadcast([128, T2_USED]),
                        op=mybir.AluOpType.mult)
                    for cch in range(kt + 1):
                        nc.tensor.matmul(out=ps2[:], lhsT=ident[:], rhs=rhs2[:, cch, :],
                                         start=(cch == 0), stop=(cch == kt))
                    rec2 = smp.tile([128, 1], F32, tag="rec2")
                    nc.vector.reciprocal(out=rec2[:], in_=ps2[:, HID + 2:HID + 3])
                    y2 = ppy.tile([128, HID], F32, tag="y")
                    nc.vector.tensor_tensor(
                        out=y2[:], in0=ps2[:, 0:HID],
                        in1=rec2[:].to_broadcast([128, HID]),
                        op=mybir.AluOpType.mult)
                    m2 = ep.tile([128, HID], F32, tag="m2")
                    nc.vector.tensor_scalar(out=m2[:], in0=y2[:], scalar1=0.0, scalar2=None,
                                            op0=mybir.AluOpType.min)
                    eE2 = ep.tile([128, HID], F32, tag="eE2")
                    nc.scalar.activation(eE2[:], m2[:], mybir.ActivationFunctionType.Exp)
                    r2 = ep.tile([128, HID], F32, tag="r2")
                    nc.vector.tensor_scalar(out=r2[:], in0=y2[:], scalar1=0.0, scalar2=-1.0,
                                            op0=mybir.AluOpType.max, op1=mybir.AluOpType.add)
                    h3 = ep.tile([128, HID], BF16, tag="h3")
                    nc.vector.tensor_tensor(out=h3[:], in0=eE2[:], in1=r2[:], op=mybir.AluOpType.add)
                    pt2 = ppm.tile([128, 128], BF16, tag="misc")
                    nc.tensor.transpose(out=pt2[:HID, :], in_=h3[:], identity=ident[:])
                    h3T = ep.tile([HID, 128], BF16, tag="h3T")
                    nc.scalar.activation(h3T[:], pt2[:HID, :], mybir.ActivationFunctionType.Copy)
                    psf = ppm.tile([128, OUT_CH], F32, tag="misc")
                    nc.tensor.matmul(out=psf[:], lhsT=h3T[:], rhs=Wout[:], start=True, stop=True)
                    outf = ep.tile([128, OUT_CH], F32, tag="outf")
                    nc.vector.tensor_tensor(out=outf[:], in0=psf[:], in1=boutr[:],
                                            op=mybir.AluOpType.add)
                    nc.sync.dma_start(out=out_d[t * 128:(t + 1) * 128, :], in_=outf[:])

    nc.compile()
    return nc


def _run(inputs, trace=False):
    meta, in_maps = _prep(**inputs)
    nc = _build(meta)
    res = run_bass_kernel_spmd(nc, in_maps, core_ids=list(range(N_CORES)), trace=trace)
    outg = np.concatenate([res.results[c]["out"] for c in range(N_CORES)], axis=0)
    out_nodes = np.empty((meta["n_pad"], OUT_CH), np.float32)
    out_nodes[meta["perm_rows"]] = outg
    return out_nodes[:meta["N"]], res


def kernel(**inputs):
    out, _ = _run(inputs, trace=False)
    return out


# revision 21
# speedup vs baseline: 2.0875x; 1.0438x over previous
"""GAT (2-layer, 4-head then 1-head) on 8 Trainium2 NeuronCores.

Strategy
--------
- Nodes are permuted: globally degree-sorted, dealt round-robin to 8 cores
  (edge balance + nearly-identical degree profiles per core), then each
  core's nodes are laid out in 128-node dst tiles. Tiles are degree-uniform,
  so per-dst edge lists pad to the tile max with small waste.
- Edges land in a "slot grid" [128 dst x K slots] per tile: slot-chunk c is
  128 edges whose partition IS the dst row. The aggregation matmul then has
  an identity stationary operand.
- Per-edge messages are fetched with dma_gather (int16 indices) issued as
  prepare_only descriptors + trigger_dma so transfers run async on 4 SWDGE
  queues. The bottleneck is Q7 descriptor generation (~8ns/idx), so the
  index count is minimized:
    * self-loops are NOT gathered: each tile's self contribution is built
      locally as one extra rhs chunk (w_self * [h|1]).
    * the int16 range split uses OVERLAPPING views (A = rows 0..32767 from
      base 0, B = rows 8194..40961 from base 8194). Edges with src row in
      [8194, 32767] can go to either side and are assigned to balance the
      per-tile (Ka, Kb) caps, nearly halving the padding.
- Sentinel rows (0 and last) have attention logits of -60000 so padded
  slots get weight exp(-inf)=0 and contribute nothing.
- Softmax without max-subtraction: w_e = exp(leakyrelu(x)) = max(exp(x),
  exp(0.2 x)), accumulated per dst in PSUM along with the denominator
  (ones-column trick), divided once per node.
- Layer outputs are transformed (W2 / W_out) per tile; node tables are
  AllGather'd across cores between layers.
"""

import numpy as np

import concourse.bacc as bacc
import concourse.mybir as mybir
import concourse.tile as tile
from concourse.bass_utils import run_bass_kernel_spmd

F32 = mybir.dt.float32
BF16 = mybir.dt.float16  # NB: fp16 (renamed var kept)
I16 = mybir.dt.int16

IN_CH = 128
HID = 32
HEADS = 4
OUT_CH = 112
NEG_SLOPE = 0.2
NEG_BIG = -60000.0

# Layer-1 table row: [h(128)] = 128 bf16 (256B); al_src is recomputed on-device
T1_COLS = 128
T1_OWN = T1_COLS
# Layer-2 table row: [z(32) | as2(1) | ad2(1) | one(1) | pad] = 128 bf16 (256B)
T2_COLS = 128
T2_USED = 35
T2_OWN = T2_COLS

N_CORES = 8
GCAP = 56
N_AG_CHUNKS = 4


def _prep(x, edge_index, W1, a_src1, a_dst1, b1, W2, a_src2, a_dst2, b2, W_out, b_out):
    """Host-side graph preprocessing. Returns (meta, per-core inputs)."""
    N = x.shape[0]
    per_core = -(-N // (N_CORES * 128)) * 128
    n_pad = per_core * N_CORES
    NT = per_core // 128
    nrows = n_pad + 2  # + 2 sentinel rows
    BB = nrows - 1 - 32767  # first row of the B view; B covers rows BB..nrows-1
    assert BB >= 0 and BB <= 32767

    src = np.asarray(edge_index[0], np.int64)
    dst = np.asarray(edge_index[1], np.int64)
    E2 = src.shape[0]

    deg_s = np.bincount(dst, minlength=n_pad) + 1  # incl self loop, for sorting
    order = np.argsort(deg_s, kind="stable")  # ascending degree, pads first
    rank = np.empty(n_pad, np.int64)
    rank[order] = np.arange(n_pad)
    core_of = rank % N_CORES
    pos_in_core = rank // N_CORES
    grow = core_of * per_core + pos_in_core      # local row (core-major, for dst/output)
    CHK = per_core // N_AG_CHUNKS
    chunk = pos_in_core // CHK
    pic = pos_in_core % CHK
    # table row order is chunk-major so each AllGather chunk lands contiguously
    trow = 1 + chunk * (N_CORES * CHK) + core_of * CHK + pic
    perm_rows = np.empty(n_pad, np.int64)
    perm_rows[grow] = np.arange(n_pad)

    sr = trow[src]
    dr = grow[dst]
    # class: 0 = A-only (row < BB), 1 = flex, 2 = B-only (row > 32767)
    cls = np.where(sr < BB, 0, np.where(sr > 32767, 2, 1)).astype(np.int64)

    deg = np.bincount(dr, minlength=n_pad)          # per dst row, no self
    nA = np.bincount(dr[cls == 0], minlength=n_pad)
    nB = np.bincount(dr[cls == 2], minlength=n_pad)
    tile_of = (np.arange(n_pad) % per_core) // 128

    Ka = np.zeros(NT, np.int64)
    Kb = np.zeros(NT, np.int64)
    for t in range(NT):
        sel = tile_of == t
        ka = nA[sel].max()
        kb = nB[sel].max()
        ka += max(0, deg[sel].max() - ka - kb)  # ensure Ka+Kb >= maxdeg
        Ka[t], Kb[t] = ka, kb
    Kt = Ka + Kb

    # per-dst A-count: a = max(nA, deg - Kb[tile])
    a_of = np.maximum(nA, deg - Kb[tile_of])

    # sort edges by (dst row, class); first a_of[d] edges of each run -> A
    eorder = np.lexsort((cls, dr))
    dr_s = dr[eorder]
    sr_s = sr[eorder]
    newrun = np.empty(E2, bool)
    newrun[0] = True
    newrun[1:] = dr_s[1:] != dr_s[:-1]
    run_start_idx = np.flatnonzero(newrun)
    run_id = np.cumsum(newrun) - 1
    pos_in_run = np.arange(E2) - run_start_idx[run_id]
    in_A = pos_in_run < a_of[dr_s]
    # slot within its side
    slot = np.where(in_A, pos_in_run, pos_in_run - a_of[dr_s])

    # adaptive groups: consecutive tiles, sum of slots <= GCAP
    groups = []
    t = 0
    while t < NT:
        e = t
        tot = 0
        while e < NT and (e == t or tot + Kt[e] <= GCAP):
            tot += Kt[e]
            e += 1
        groups.append((t, e))
        t = e
    base_a = np.zeros(NT, np.int64)
    base_b = np.zeros(NT, np.int64)
    ga_base = []
    gb_base = []
    off = 0
    for (t0, t1) in groups:
        ga_base.append(off)
        for t in range(t0, t1):
            base_a[t] = off
            off += 128 * Ka[t]
        gb_base.append(off)
        for t in range(t0, t1):
            base_b[t] = off
            off += 128 * Kb[t]
    totidx = off

    # default stream = sentinels (A: row 0; B: local 32767 = last row)
    default = np.zeros(totidx, np.int16)
    for t in range(NT):
        default[base_a[t]:base_a[t] + 128 * Ka[t]] = 0
        default[base_b[t]:base_b[t] + 128 * Kb[t]] = 32767
    streams = np.tile(default, (N_CORES, 1))

    e_core = dr_s // per_core
    loc = dr_s % per_core
    tl = loc // 128
    p = loc % 128
    posA = base_a[tl] + slot * 128 + p
    posB = base_b[tl] + slot * 128 + p
    pos = np.where(in_A, posA, posB)
    val = np.where(in_A, sr_s, sr_s - BB).astype(np.int16)
    assert val.min() >= 0
    streams[e_core, pos] = val

    # pad mask: 1.0 where a real edge occupies the slot cell
    mask = np.zeros((N_CORES, 128, totidx // 128), np.float16)
    mask[e_core, p, pos // 128] = 1.0

    # wrap for dma_gather: wrapped[p, j] = flat[j*16 + p%16]
    assert totidx % 16 == 0
    idx_wrapped = np.empty((N_CORES, 128, totidx // 16), np.int16)
    for c in range(N_CORES):
        w16 = streams[c].reshape(-1, 16).T  # [16, totidx/16]
        idx_wrapped[c] = np.tile(w16, (8, 1))

    # x slices (table-row order per core), fp16 for fast PE loads
    xp = np.zeros((n_pad, IN_CH), np.float32)
    xp[:N] = np.asarray(x, np.float32)
    x_slices = np.empty((N_CORES, IN_CH, per_core), np.float16)
    for c in range(N_CORES):
        nodes = perm_rows[c * per_core:(c + 1) * per_core]
        x_slices[c] = xp[nodes].T.astype(np.float16)

    # weight packs
    W1 = np.asarray(W1, np.float32)
    Bsrc = np.zeros((HEADS * HID, HEADS), np.float32)
    Bdst = np.zeros((HEADS * HID, HEADS), np.float32)
    for h in range(HEADS):
        Bsrc[h * HID:(h + 1) * HID, h] = np.asarray(a_src1[h], np.float32)
        Bdst[h * HID:(h + 1) * HID, h] = np.asarray(a_dst1[h], np.float32)
    W1big = np.concatenate([W1, W1 @ Bsrc, W1 @ Bdst], axis=1)  # [128, 136]
    asrc_rep = np.tile(W1big[0:1, 0:0], (128, 1))  # placeholder
    asrc_flat = np.asarray(a_src1, np.float32).reshape(1, HEADS * HID)
    asrc_rep = np.tile(asrc_flat, (128, 1))  # [128, 128] same row per partition
    W2 = np.asarray(W2, np.float32)
    W2big = np.concatenate(
        [W2, W2 @ np.asarray(a_src2, np.float32).T, W2 @ np.asarray(a_dst2, np.float32).T],
        axis=1,
    )  # [128, 34]
    b1v = np.asarray(b1, np.float32).reshape(HEADS, HID)
    b1i = np.zeros((HEADS, HID + 1), np.float32)
    b1i[:, :HID] = b1v
    b1_rep = np.tile(b1i.reshape(1, -1), (128, 1))                            # [128,132]
    b2_rep = np.zeros((128, HID + 2), np.float32)
    b2_rep[:, :HID] = np.asarray(b2, np.float32)[None, :]
    bout_rep = np.tile(np.asarray(b_out, np.float32)[None, :], (128, 1))     # [128,112]
    ident = np.eye(128, dtype=np.float32)

    bf16 = np.float16
    sent1 = np.zeros((1, T1_COLS), bf16)
    sent2 = np.zeros((1, 36), bf16)
    sent2[0, 32:34] = NEG_BIG

    meta = dict(
        N=N, E2=E2, n_pad=n_pad, per_core=per_core, NT=NT, nrows=nrows,
        b_base=BB, Ka=Ka.tolist(), Kb=Kb.tolist(),
        base_a=base_a.tolist(), base_b=base_b.tolist(), totidx=totidx,
        ga_base=ga_base, gb_base=gb_base, groups=groups,
        perm_rows=perm_rows,
    )
    shared = dict(
        W1big=W1big.astype(bf16), W2big=W2big.astype(bf16), asrc_rep=asrc_rep.astype(bf16),
        Wout=np.asarray(W_out, np.float32).astype(bf16),
        b1_rep=b1_rep, b2_rep=b2_rep, bout_rep=bout_rep, ident=ident.astype(bf16),
        sent1=sent1, sent2=sent2,
    )
    in_maps = []
    for c in range(N_CORES):
        m = dict(shared)
        m["x_slice"] = np.ascontiguousarray(x_slices[c])
        m["idx_flat"] = np.ascontiguousarray(idx_wrapped[c])
        m["edge_mask"] = np.ascontiguousarray(mask[c])
        in_maps.append(m)
    return meta, in_maps


def _build(meta):
    per_core, NT, nrows, b_base = meta["per_core"], meta["NT"], meta["nrows"], meta["b_base"]
    Ka, Kb = meta["Ka"], meta["Kb"]
    base_a, base_b, totidx = meta["base_a"], meta["base_b"], meta["totidx"]
    ga_base, gb_base, groups = meta["ga_base"], meta["gb_base"], meta["groups"]

    nc = bacc.Bacc("TRN2", num_devices=N_CORES, num_swdge_queues=4,
                   dynamic_dma_scratch_size=65536)

    x_slice = nc.dram_tensor("x_slice", [IN_CH, per_core], BF16, kind="ExternalInput")
    idx_flat = nc.dram_tensor("idx_flat", [128, totidx // 16], I16, kind="ExternalInput")
    W1big_d = nc.dram_tensor("W1big", [128, 136], BF16, kind="ExternalInput")
    asrc_d = nc.dram_tensor("asrc_rep", [128, 128], BF16, kind="ExternalInput")
    mask_d = nc.dram_tensor("edge_mask", [128, totidx // 128], BF16, kind="ExternalInput")
    W2big_d = nc.dram_tensor("W2big", [128, HID + 2], BF16, kind="ExternalInput")
    Wout_d = nc.dram_tensor("Wout", [HID, OUT_CH], BF16, kind="ExternalInput")
    b1_d = nc.dram_tensor("b1_rep", [128, 132], F32, kind="ExternalInput")
    b2_d = nc.dram_tensor("b2_rep", [128, HID + 2], F32, kind="ExternalInput")
    bout_d = nc.dram_tensor("bout_rep", [128, OUT_CH], F32, kind="ExternalInput")
    ident_d = nc.dram_tensor("ident", [128, 128], BF16, kind="ExternalInput")
    sent1_d = nc.dram_tensor("sent1", [1, T1_COLS], BF16, kind="ExternalInput")
    sent2_d = nc.dram_tensor("sent2", [1, 36], BF16, kind="ExternalInput")

    CHK = per_core // N_AG_CHUNKS
    TPC = NT // N_AG_CHUNKS  # tiles per AG chunk
    T1_ownc = [nc.dram_tensor(f"T1_own{j}", [CHK, T1_OWN], BF16, kind="Internal")
               for j in range(N_AG_CHUNKS)]
    T1_sh = nc.dram_tensor("T1_sh", [nrows, T1_COLS], BF16, kind="Internal", addr_space="Shared")
    T2_ownc = [nc.dram_tensor(f"T2_own{j}", [CHK, T2_OWN], BF16, kind="Internal")
               for j in range(N_AG_CHUNKS)]
    T2_sh = nc.dram_tensor("T2_sh", [nrows, T2_COLS], BF16, kind="Internal", addr_space="Shared")
    out_d = nc.dram_tensor("out", [per_core, OUT_CH], F32, kind="ExternalOutput")

    rgroups = [list(range(N_CORES))]
    qctr = [0]
    qsems = [nc.alloc_semaphore(f"gsem{q}") for q in range(4)]

    def qn():
        q = qctr[0] % 4
        qctr[0] += 1
        return q

    import os
    ASYNC = os.environ.get("GAT_ASYNC_GATHER", "0") == "1"

    def async_gather(out_ap, table_ap, idx_ap, nidx, cols):
        """Prep descriptors on a rotating SWDGE queue; fire the DMA async."""
        q = qn()
        if ASYNC:
            nc.gpsimd.dma_gather(
                out_ap, table_ap, idx_ap, nidx, nidx, cols,
                prepare_only=True, sem=qsems[q], queue_num=q, single_packet=False)
            nc.gpsimd.trigger_dma(count=None, queue_num=q)
        else:
            nc.gpsimd.dma_gather(
                out_ap, table_ap, idx_ap, nidx, nidx, cols,
                queue_num=q, single_packet=False)

    with tile.TileContext(nc) as tc:
        with (
            tc.tile_pool(name="const", bufs=1) as cp,
            tc.tile_pool(name="xa", bufs=2) as xap,
            tc.tile_pool(name="stage", bufs=3) as sp,
            tc.tile_pool(name="g1", bufs=2) as g1p,
            tc.tile_pool(name="g2", bufs=2) as g2p,
            tc.tile_pool(name="small", bufs=4) as smp,
            tc.tile_pool(name="rhs", bufs=2) as rp,
            tc.tile_pool(name="epi", bufs=3) as ep,
            tc.tile_pool(name="psa", bufs=3, space="PSUM") as ppa,
            tc.tile_pool(name="psm", bufs=3, space="PSUM") as ppm,
            tc.tile_pool(name="psy", bufs=2, space="PSUM") as ppy,
        ):
            # ---- consts to SBUF
            W1big = cp.tile([128, 136], BF16)
            nc.sync.dma_start(out=W1big[:], in_=W1big_d[:])
            asrc = cp.tile([128, 128], BF16)
            nc.sync.dma_start(out=asrc[:], in_=asrc_d[:])
            maskt = cp.tile([128, totidx // 128], BF16)
            nc.sync.dma_start(out=maskt[:], in_=mask_d[:])
            W2big = cp.tile([128, HID + 2], BF16)
            nc.sync.dma_start(out=W2big[:], in_=W2big_d[:])
            Wout = cp.tile([HID, OUT_CH], BF16)
            nc.sync.dma_start(out=Wout[:], in_=Wout_d[:])
            b1r = cp.tile([128, 132], F32)
            nc.sync.dma_start(out=b1r[:], in_=b1_d[:])
            b2r = cp.tile([128, HID + 2], F32)
            nc.sync.dma_start(out=b2r[:], in_=b2_d[:])
            boutr = cp.tile([128, OUT_CH], F32)
            nc.sync.dma_start(out=boutr[:], in_=bout_d[:])
            ident = cp.tile([128, 128], BF16)
            nc.sync.dma_start(out=ident[:], in_=ident_d[:])
            idxs = cp.tile([128, totidx // 16], I16)
            nc.sync.dma_start(out=idxs[:], in_=idx_flat[:])
            # SBUF-resident own-node data (no DRAM roundtrip)
            t1d_sb = cp.tile([128, NT * 8], BF16)    # [al_src(4)|al_dst(4)] per tile
            t2d_sb = cp.tile([128, NT * 2], BF16)    # [as2|ad2] per tile
            hball = cp.tile([128, NT * 132], BF16)   # own [h|1]x4 rows per tile
            nc.vector.memset(hball[:], 1.0)
            zball = cp.tile([128, NT * T2_USED], BF16)  # own [z|as2|ad2|1] per tile

            # ---- phase A: own node tiles -> T1_own
            for t in range(NT):
                xa = xap.tile([128, 128], BF16)
                nc.sync.dma_start(out=xa[:], in_=x_slice[:, t * 128:(t + 1) * 128])
                ps = ppa.tile([128, 136], F32, tag="agg")
                nc.tensor.matmul(out=ps[:], lhsT=xa[:], rhs=W1big[:], start=True, stop=True)
                hb = sp.tile([128, T1_COLS], BF16)
                nc.vector.tensor_copy(out=hb[:], in_=ps[:, 0:128])
                lt = (t % TPC) * 128
                nc.sync.dma_start(out=T1_ownc[t // TPC][lt:lt + 128, :], in_=hb[:])
                nc.scalar.activation(t1d_sb[:, t * 8:(t + 1) * 8], ps[:, 128:136],
                                     mybir.ActivationFunctionType.Copy)
                hball_v = hball[:, t * 132:(t + 1) * 132].rearrange("p (h j) -> p h j", h=HEADS)
                nc.scalar.activation(hball_v[:, :, 0:HID],
                                     ps[:, 0:128].rearrange("p (h c) -> p h c", h=HEADS),
                                     mybir.ActivationFunctionType.Copy)

            # ---- allgather T1 (chunked; chunk j fires when its tiles are stored)
            for j in range(N_AG_CHUNKS):
                o = 1 + j * CHK * N_CORES
                nc.gpsimd.collective_compute(
                    "AllGather", mybir.AluOpType.bypass, replica_groups=rgroups,
                    ins=[T1_ownc[j][:]], outs=[T1_sh[o:o + CHK * N_CORES, :]],
                )
            s1 = cp.tile([1, T1_COLS], BF16)
            nc.sync.dma_start(out=s1[:], in_=sent1_d[:])
            nc.sync.dma_start(out=T1_sh[0:1, :], in_=s1[:])
            nc.sync.dma_start(out=T1_sh[nrows - 1:nrows, :], in_=s1[:])

            # ---- layer 1 edge phase (grouped super-gathers)
            for gi, (t0, t1) in enumerate(groups):
                tiles = list(range(t0, t1))
                SA = sum(Ka[t] for t in tiles)
                SB = sum(Kb[t] for t in tiles)
                GA = g1p.tile([128, max(SA, 1), T1_COLS], BF16, tag="GA")
                if SA:
                    o = ga_base[gi]
                    async_gather(GA[:, 0:SA, :], T1_sh[:],
                                 idxs[:, o // 16: o // 16 + SA * 8], 128 * SA, T1_COLS)
                GB = g1p.tile([128, max(SB, 1), T1_COLS], BF16, tag="GB")
                if SB:
                    o = gb_base[gi]
                    async_gather(GB[:, 0:SB, :], T1_sh[b_base:nrows, :],
                                 idxs[:, o // 16: o // 16 + SB * 8], 128 * SB, T1_COLS)
                for t in tiles:
                    ka, kb = Ka[t], Kb[t]
                    kt = ka + kb
                    aoff = (base_a[t] - ga_base[gi]) // 128
                    boff = (base_b[t] - gb_base[gi]) // 128
                    ps = ppa.tile([128, HEADS * (HID + 1)], F32, tag="agg")
                    rhs = rp.tile([128, kt + 1, HEADS * (HID + 1)], BF16, tag="rhs1")
                    rhs_v = rhs[:].rearrange("p k (h j) -> p k h j", h=HEADS)
                    for (G, goff, nk, ro, mbase) in (
                            (GA, aoff, ka, 0, base_a[t] // 128),
                            (GB, boff, kb, ka, base_b[t] // 128)):
                        if nk == 0:
                            continue
                        Gs = G[:, goff:goff + nk, :]
                        Gh = Gs[:].rearrange("p k (h c) -> p k h c", h=HEADS)
                        # recompute al_src = sum_c h * a_src
                        tmp = smp.tile([128, nk, 128], BF16, tag="tmp")
                        nc.vector.tensor_tensor(
                            out=tmp[:], in0=Gs[:],
                            in1=asrc[:, None, :].to_broadcast([128, nk, 128]),
                            op=mybir.AluOpType.mult)
                        asl = smp.tile([128, nk, HEADS], F32, tag="asl")
                        nc.vector.tensor_reduce(
                            out=asl[:],
                            in_=tmp[:].rearrange("p k (h c) -> p k h c", h=HEADS),
                            axis=mybir.AxisListType.X, op=mybir.AluOpType.add)
                        xl = smp.tile([128, nk, HEADS], BF16, tag="xl")
                        nc.vector.tensor_tensor(
                            out=xl[:], in0=asl[:],
                            in1=t1d_sb[:, None, t * 8 + 4:t * 8 + 8].to_broadcast([128, nk, HEADS]),
                            op=mybir.AluOpType.add)
                        e1 = smp.tile([128, nk, HEADS], BF16, tag="e1")
                        nc.scalar.activation(e1[:], xl[:], mybir.ActivationFunctionType.Exp)
                        e2 = smp.tile([128, nk, HEADS], BF16, tag="e2")
                        nc.scalar.activation(e2[:], xl[:], mybir.ActivationFunctionType.Exp, scale=NEG_SLOPE)
                        wu = smp.tile([128, nk, HEADS], BF16, tag="wu")
                        nc.vector.tensor_tensor(out=wu[:], in0=e1[:], in1=e2[:], op=mybir.AluOpType.max)
                        # zero out pad slots
                        w = smp.tile([128, nk, HEADS], BF16, tag="w")
                        nc.vector.tensor_tensor(
                            out=w[:], in0=wu[:],
                            in1=maskt[:, mbase:mbase + nk, None].to_broadcast([128, nk, HEADS]),
                            op=mybir.AluOpType.mult)
                        nc.vector.tensor_tensor(
                            out=rhs_v[:, ro:ro + nk, :, 0:HID],
                            in0=Gh,
                            in1=w[:, :, :, None].to_broadcast([128, nk, HEADS, HID]),
                            op=mybir.AluOpType.mult)
                        nc.scalar.activation(rhs_v[:, ro:ro + nk, :, HID], w[:],
                                             mybir.ActivationFunctionType.Copy)
                    # self-loop chunk: w_self * [h|1]
                    xls = smp.tile([128, HEADS], BF16, tag="xls")
                    nc.vector.tensor_tensor(
                        out=xls[:], in0=t1d_sb[:, t * 8:t * 8 + 4],
                        in1=t1d_sb[:, t * 8 + 4:t * 8 + 8], op=mybir.AluOpType.add)
                    e1s = smp.tile([128, HEADS], BF16, tag="e1s")
                    nc.scalar.activation(e1s[:], xls[:], mybir.ActivationFunctionType.Exp)
                    e2s = smp.tile([128, HEADS], BF16, tag="e2s")
                    nc.scalar.activation(e2s[:], xls[:], mybir.ActivationFunctionType.Exp, scale=NEG_SLOPE)
                    ws = smp.tile([128, HEADS], BF16, tag="ws")
                    nc.vector.tensor_tensor(out=ws[:], in0=e1s[:], in1=e2s[:], op=mybir.AluOpType.max)
                    nc.vector.tensor_tensor(
                        out=rhs_v[:, kt, :, :],
                        in0=hball[:, t * 132:(t + 1) * 132].rearrange("p (h j) -> p h j", h=HEADS),
                        in1=ws[:, :, None].to_broadcast([128, HEADS, HID + 1]),
                        op=mybir.AluOpType.mult)
                    for cch in range(kt + 1):
                        nc.tensor.matmul(out=ps[:], lhsT=ident[:], rhs=rhs[:, cch, :],
                                         start=(cch == 0), stop=(cch == kt))
                    # epilogue: divide, +b1, ELU
                    ps_v = ps[:].rearrange("p (h j) -> p h j", h=HEADS)
                    rec = smp.tile([128, HEADS], F32, tag="rec")
                    nc.vector.reciprocal(out=rec[:], in_=ps_v[:, :, HID])
                    y1 = ppy.tile([128, 128], F32, tag="y")
                    nc.vector.tensor_tensor(
                        out=y1[:].rearrange("p (h j) -> p h j", h=HEADS),
                        in0=ps_v[:, :, 0:HID],
                        in1=rec[:, :, None].to_broadcast([128, HEADS, HID]),
                        op=mybir.AluOpType.mult)
                    nc.vector.tensor_tensor(
                        out=y1[:].rearrange("p (h j) -> p h j", h=HEADS),
                        in0=y1[:].rearrange("p (h j) -> p h j", h=HEADS),
                        in1=b1r[:].rearrange("p (h j) -> p h j", h=HEADS)[:, :, 0:HID],
                        op=mybir.AluOpType.add)
                    m1 = ep.tile([128, 128], F32, tag="m1")
                    nc.vector.tensor_scalar(out=m1[:], in0=y1[:], scalar1=0.0, scalar2=None,
                                            op0=mybir.AluOpType.min)
                    eE = ep.tile([128, 128], F32, tag="eE")
                    nc.scalar.activation(eE[:], m1[:], mybir.ActivationFunctionType.Exp)
                    r1 = ep.tile([128, 128], F32, tag="r1")
                    nc.vector.tensor_scalar(out=r1[:], in0=y1[:], scalar1=0.0, scalar2=-1.0,
                                            op0=mybir.AluOpType.max, op1=mybir.AluOpType.add)
                    h2 = ep.tile([128, 128], BF16, tag="h2")
                    nc.vector.tensor_tensor(out=h2[:], in0=eE[:], in1=r1[:], op=mybir.AluOpType.add)
                    # transpose h2, z = h2 @ W2big
                    pt = ppm.tile([128, 128], BF16, tag="misc")
                    nc.tensor.transpose(out=pt[:], in_=h2[:], identity=ident[:])
                    h2T = ep.tile([128, 128], BF16, tag="h2T")
                    nc.scalar.activation(h2T[:], pt[:], mybir.ActivationFunctionType.Copy)
                    psz = ppm.tile([128, HID + 2], F32, tag="misc")
                    nc.tensor.matmul(out=psz[:], lhsT=h2T[:], rhs=W2big[:], start=True, stop=True)
                    t2b = sp.tile([128, T2_OWN], BF16, tag="t2b")
                    nc.vector.tensor_tensor(out=t2b[:, 0:HID + 2], in0=psz[:], in1=b2r[:],
                                            op=mybir.AluOpType.add)
                    nc.vector.memset(t2b[:, HID + 2:HID + 3], 1.0)
                    lt2 = (t % TPC) * 128
                    nc.sync.dma_start(out=T2_ownc[t // TPC][lt2:lt2 + 128, :], in_=t2b[:])
                    nc.scalar.activation(zball[:, t * T2_USED:(t + 1) * T2_USED],
                                         t2b[:, 0:T2_USED], mybir.ActivationFunctionType.Copy)
                    nc.scalar.activation(t2d_sb[:, t * 2:(t + 1) * 2], t2b[:, 32:34],
                                         mybir.ActivationFunctionType.Copy)

            # ---- allgather T2 (chunked) + sentinel pokes
            for j in range(N_AG_CHUNKS):
                o = 1 + j * CHK * N_CORES
                nc.gpsimd.collective_compute(
                    "AllGather", mybir.AluOpType.bypass, replica_groups=rgroups,
                    ins=[T2_ownc[j][:]], outs=[T2_sh[o:o + CHK * N_CORES, :]],
                )
            s2 = cp.tile([1, 36], BF16)
            nc.sync.dma_start(out=s2[:], in_=sent2_d[:])
            nc.sync.dma_start(out=T2_sh[0:1, 0:36], in_=s2[:])
            nc.sync.dma_start(out=T2_sh[nrows - 1:nrows, 0:36], in_=s2[:])

            # ---- layer 2 edge phase + output (grouped super-gathers)
            for gi, (t0, t1) in enumerate(groups):
                tiles2 = list(range(t0, t1))
                SA = sum(Ka[t] for t in tiles2)
                SB = sum(Kb[t] for t in tiles2)
                GA2 = g2p.tile([128, max(SA, 1), T2_COLS], BF16, tag="GA2")
                if SA:
                    o = ga_base[gi]
                    async_gather(GA2[:, 0:SA, :], T2_sh[:],
                                 idxs[:, o // 16: o // 16 + SA * 8], 128 * SA, T2_COLS)
                GB2 = g2p.tile([128, max(SB, 1), T2_COLS], BF16, tag="GB2")
                if SB:
                    o = gb_base[gi]
                    async_gather(GB2[:, 0:SB, :], T2_sh[b_base:nrows, :],
                                 idxs[:, o // 16: o // 16 + SB * 8], 128 * SB, T2_COLS)
                for t in tiles2:
                    ka, kb = Ka[t], Kb[t]
                    kt = ka + kb
                    aoff = (base_a[t] - ga_base[gi]) // 128
                    boff = (base_b[t] - gb_base[gi]) // 128
                    ps2 = ppa.tile([128, T2_USED], F32, tag="agg")
                    rhs2 = rp.tile([128, kt + 1, T2_USED], BF16, tag="rhs2")
                    for (G2, goff, nk, ro) in ((GA2, aoff, ka, 0), (GB2, boff, kb, ka)):
                        if nk == 0:
                            continue
                        Gs = G2[:, goff:goff + nk, :]
                        xl2 = smp.tile([128, nk, 1], BF16, tag="xl2")
                        nc.vector.tensor_tensor(
                            out=xl2[:], in0=Gs[:, :, 32:33],
                            in1=t2d_sb[:, None, t * 2 + 1:t * 2 + 2].to_broadcast([128, nk, 1]),
                            op=mybir.AluOpType.add)
                        e1b = smp.tile([128, nk, 1], BF16, tag="e1b")
                        nc.scalar.activation(e1b[:], xl2[:], mybir.ActivationFunctionType.Exp)
                        e2b = smp.tile([128, nk, 1], BF16, tag="e2b")
                        nc.scalar.activation(e2b[:], xl2[:], mybir.ActivationFunctionType.Exp, scale=NEG_SLOPE)
                        w2 = smp.tile([128, nk, 1], BF16, tag="w2")
                        nc.vector.tensor_tensor(out=w2[:], in0=e1b[:], in1=e2b[:], op=mybir.AluOpType.max)
                        nc.vector.tensor_tensor(
                            out=rhs2[:, ro:ro + nk, :],
                            in0=Gs[:, :, 0:T2_USED],
                            in1=w2[:, :, :].to_broadcast([128, nk, T2_USED]),
                            op=mybir.AluOpType.mult)
                    # self-loop chunk
                    xl2s = smp.tile([128, 1], BF16, tag="xl2s")
                    nc.vector.tensor_tensor(
                        out=xl2s[:], in0=t2d_sb[:, t * 2:t * 2 + 1],
                        in1=t2d_sb[:, t * 2 + 1:t * 2 + 2], op=mybir.AluOpType.add)
                    e1bs = smp.tile([128, 1], BF16, tag="e1bs")
                    nc.scalar.activation(e1bs[:], xl2s[:], mybir.ActivationFunctionType.Exp)
                    e2bs = smp.tile([128, 1], BF16, tag="e2bs")
                    nc.scalar.activation(e2bs[:], xl2s[:], mybir.ActivationFunctionType.Exp, scale=NEG_SLOPE)
                    w2s = smp.tile([128, 1], BF16, tag="w2s")
                    nc.vector.tensor_tensor(out=w2s[:], in0=e1bs[:], in1=e2bs[:], op=mybir.AluOpType.max)
                    nc.vector.tensor_tensor(
                        out=rhs2[:, kt, :],
                        in0=zball[:, t * T2_USED:(t + 1) * T2_USED],
                        in1=w2s[:].to_broadcast([128, T2_USED]),
                        op=mybir.AluOpType.mult)
                    for cch in range(kt + 1):
                        nc.tensor.matmul(out=ps2[:], lhsT=ident[:], rhs=rhs2[:, cch, :],
                                         start=(cch == 0), stop=(cch == kt))
                    rec2 = smp.tile([128, 1], F32, tag="rec2")
                    nc.vector.reciprocal(out=rec2[:], in_=ps2[:, HID + 2:HID + 3])
                    y2 = ppy.tile([128, HID], F32, tag="y")
                    nc.vector.tensor_tensor(
                        out=y2[:], in0=ps2[:, 0:HID],
                        in1=rec2[:].to_broadcast([128, HID]),
                        op=mybir.AluOpType.mult)
                    m2 = ep.tile([128, HID], F32, tag="m2")
                    nc.vector.tensor_scalar(out=m2[:], in0=y2[:], scalar1=0.0, scalar2=None,
                                            op0=mybir.AluOpType.min)
                    eE2 = ep.tile([128, HID], F32, tag="eE2")
                    nc.scalar.activation(eE2[:], m2[:], mybir.ActivationFunctionType.Exp)
                    r2 = ep.tile([128, HID], F32, tag="r2")
                    nc.vector.tensor_scalar(out=r2[:], in0=y2[:], scalar1=0.0, scalar2=-1.0,
                                            op0=mybir.AluOpType.max, op1=mybir.AluOpType.add)
                    h3 = ep.tile([128, HID], BF16, tag="h3")
                    nc.vector.tensor_tensor(out=h3[:], in0=eE2[:], in1=r2[:], op=mybir.AluOpType.add)
                    pt2 = ppm.tile([128, 128], BF16, tag="misc")
                    nc.tensor.transpose(out=pt2[:HID, :], in_=h3[:], identity=ident[:])
                    h3T = ep.tile([HID, 128], BF16, tag="h3T")
                    nc.scalar.activation(h3T[:], pt2[:HID, :], mybir.ActivationFunctionType.Copy)
                    psf = ppm.tile([128, OUT_CH], F32, tag="misc")
                    nc.tensor.matmul(out=psf[:], lhsT=h3T[:], rhs=Wout[:], start=True, stop=True)
                    outf = ep.tile([128, OUT_CH], F32, tag="outf")
                    nc.vector.tensor_tensor(out=outf[:], in0=psf[:], in1=boutr[:],
                                            op=mybir.AluOpType.add)
                    nc.sync.dma_start(out=out_d[t * 128:(t + 1) * 128, :], in_=outf[:])

    nc.compile()
    return nc


def _run(inputs, trace=False):
    meta, in_maps = _prep(**inputs)
    nc = _build(meta)
    res = run_bass_kernel_spmd(nc, in_maps, core_ids=list(range(N_CORES)), trace=trace)
    outg = np.concatenate([res.results[c]["out"] for c in range(N_CORES)], axis=0)
    out_nodes = np.empty((meta["n_pad"], OUT_CH), np.float32)
    out_nodes[meta["perm_rows"]] = outg
    return out_nodes[:meta["N"]], res


def kernel(**inputs):
    out, _ = _run(inputs, trace=False)
    return out


# revision 25
# speedup vs baseline: 2.1768x; 1.0428x over previous
"""GAT (2-layer, 4-head then 1-head) on 8 Trainium2 NeuronCores.

Strategy
--------
- Nodes are permuted: globally degree-sorted, dealt round-robin to 8 cores
  (edge balance + nearly-identical degree profiles per core), then each
  core's nodes are laid out in 128-node dst tiles. Tiles are degree-uniform,
  so per-dst edge lists pad to the tile max with small waste.
- Edges land in a "slot grid" [128 dst x K slots] per tile: slot-chunk c is
  128 edges whose partition IS the dst row. The aggregation matmul then has
  an identity stationary operand.
- Per-edge messages are fetched with dma_gather (int16 indices) issued as
  prepare_only descriptors + trigger_dma so transfers run async on 4 SWDGE
  queues. The bottleneck is Q7 descriptor generation (~8ns/idx), so the
  index count is minimized:
    * self-loops are NOT gathered: each tile's self contribution is built
      locally as one extra rhs chunk (w_self * [h|1]).
    * the int16 range split uses OVERLAPPING views (A = rows 0..32767 from
      base 0, B = rows 8194..40961 from base 8194). Edges with src row in
      [8194, 32767] can go to either side and are assigned to balance the
      per-tile (Ka, Kb) caps, nearly halving the padding.
- Sentinel rows (0 and last) have attention logits of -60000 so padded
  slots get weight exp(-inf)=0 and contribute nothing.
- Softmax without max-subtraction: w_e = exp(leakyrelu(x)) = max(exp(x),
  exp(0.2 x)), accumulated per dst in PSUM along with the denominator
  (ones-column trick), divided once per node.
- Layer outputs are transformed (W2 / W_out) per tile; node tables are
  AllGather'd across cores between layers.
"""

import numpy as np

import concourse.bacc as bacc
import concourse.mybir as mybir
import concourse.tile as tile
from concourse.bass_utils import run_bass_kernel_spmd

F32 = mybir.dt.float32
BF16 = mybir.dt.float16  # NB: fp16 (renamed var kept)
I16 = mybir.dt.int16

IN_CH = 128
HID = 32
HEADS = 4
OUT_CH = 112
NEG_SLOPE = 0.2
NEG_BIG = -60000.0

# Layer-1 table row: [h(128)] = 128 bf16 (256B); al_src is recomputed on-device
T1_COLS = 128
T1_OWN = T1_COLS
# Layer-2 table row: [z(32) | as2(1) | ad2(1) | one(1) | pad] = 128 bf16 (256B)
T2_COLS = 128
T2_USED = 35
T2_OWN = T2_COLS

N_CORES = 8
GCAP = 56
N_AG_CHUNKS = 4


def _prep(x, edge_index, W1, a_src1, a_dst1, b1, W2, a_src2, a_dst2, b2, W_out, b_out):
    """Host-side graph preprocessing. Returns (meta, per-core inputs)."""
    N = x.shape[0]
    per_core = -(-N // (N_CORES * 128)) * 128
    n_pad = per_core * N_CORES
    NT = per_core // 128
    nrows = n_pad + 2  # + 2 sentinel rows
    BB = nrows - 1 - 32767  # first row of the B view; B covers rows BB..nrows-1
    assert BB >= 0 and BB <= 32767

    src = np.asarray(edge_index[0], np.int64)
    dst = np.asarray(edge_index[1], np.int64)
    E2 = src.shape[0]

    deg_s = np.bincount(dst, minlength=n_pad) + 1  # incl self loop, for sorting
    order = np.argsort(deg_s, kind="stable")  # ascending degree, pads first
    rank = np.empty(n_pad, np.int64)
    rank[order] = np.arange(n_pad)
    core_of = rank % N_CORES
    pos_in_core = rank // N_CORES
    grow = core_of * per_core + pos_in_core      # local row (core-major, for dst/output)
    CHK = per_core // N_AG_CHUNKS
    chunk = pos_in_core // CHK
    pic = pos_in_core % CHK
    # table row order is chunk-major so each AllGather chunk lands contiguously
    trow = 1 + chunk * (N_CORES * CHK) + core_of * CHK + pic
    perm_rows = np.empty(n_pad, np.int64)
    perm_rows[grow] = np.arange(n_pad)

    sr = trow[src]
    dr = grow[dst]
    # class: 0 = A-only (row < BB), 1 = flex, 2 = B-only (row > 32767)
    cls = np.where(sr < BB, 0, np.where(sr > 32767, 2, 1)).astype(np.int64)

    deg = np.bincount(dr, minlength=n_pad)          # per dst row, no self
    nA = np.bincount(dr[cls == 0], minlength=n_pad)
    nB = np.bincount(dr[cls == 2], minlength=n_pad)
    tile_of = (np.arange(n_pad) % per_core) // 128

    Ka = np.zeros(NT, np.int64)
    Kb = np.zeros(NT, np.int64)
    for t in range(NT):
        sel = tile_of == t
        ka = nA[sel].max()
        kb = nB[sel].max()
        ka += max(0, deg[sel].max() - ka - kb)  # ensure Ka+Kb >= maxdeg
        Ka[t], Kb[t] = ka, kb
    Kt = Ka + Kb

    # per-dst A-count: a = max(nA, deg - Kb[tile])
    a_of = np.maximum(nA, deg - Kb[tile_of])

    # sort edges by (dst row, class); first a_of[d] edges of each run -> A
    eorder = np.lexsort((cls, dr))
    dr_s = dr[eorder]
    sr_s = sr[eorder]
    newrun = np.empty(E2, bool)
    newrun[0] = True
    newrun[1:] = dr_s[1:] != dr_s[:-1]
    run_start_idx = np.flatnonzero(newrun)
    run_id = np.cumsum(newrun) - 1
    pos_in_run = np.arange(E2) - run_start_idx[run_id]
    in_A = pos_in_run < a_of[dr_s]
    # slot within its side
    slot = np.where(in_A, pos_in_run, pos_in_run - a_of[dr_s])

    # adaptive groups: consecutive tiles, sum of slots <= GCAP
    groups = []
    t = 0
    while t < NT:
        e = t
        tot = 0
        while e < NT and (e == t or tot + Kt[e] <= GCAP):
            tot += Kt[e]
            e += 1
        groups.append((t, e))
        t = e
    base_a = np.zeros(NT, np.int64)
    base_b = np.zeros(NT, np.int64)
    ga_base = []
    gb_base = []
    off = 0
    for (t0, t1) in groups:
        ga_base.append(off)
        for t in range(t0, t1):
            base_a[t] = off
            off += 128 * Ka[t]
        gb_base.append(off)
        for t in range(t0, t1):
            base_b[t] = off
            off += 128 * Kb[t]
    totidx = off

    # default stream = sentinels (A: row 0; B: local 32767 = last row)
    default = np.zeros(totidx, np.int16)
    for t in range(NT):
        default[base_a[t]:base_a[t] + 128 * Ka[t]] = 0
        default[base_b[t]:base_b[t] + 128 * Kb[t]] = 32767
    streams = np.tile(default, (N_CORES, 1))

    e_core = dr_s // per_core
    loc = dr_s % per_core
    tl = loc // 128
    p = loc % 128
    posA = base_a[tl] + slot * 128 + p
    posB = base_b[tl] + slot * 128 + p
    pos = np.where(in_A, posA, posB)
    val = np.where(in_A, sr_s, sr_s - BB).astype(np.int16)
    assert val.min() >= 0
    streams[e_core, pos] = val

    # pad mask: 1.0 where a real edge occupies the slot cell
    mask = np.zeros((N_CORES, 128, totidx // 128), np.float16)
    mask[e_core, p, pos // 128] = 1.0

    # wrap for dma_gather: wrapped[p, j] = flat[j*16 + p%16]
    assert totidx % 16 == 0
    idx_wrapped = np.empty((N_CORES, 128, totidx // 16), np.int16)
    for c in range(N_CORES):
        w16 = streams[c].reshape(-1, 16).T  # [16, totidx/16]
        idx_wrapped[c] = np.tile(w16, (8, 1))

    # x slices (table-row order per core), fp16 for fast PE loads
    xp = np.zeros((n_pad, IN_CH), np.float32)
    xp[:N] = np.asarray(x, np.float32)
    x_slices = np.empty((N_CORES, IN_CH, per_core), np.float16)
    for c in range(N_CORES):
        nodes = perm_rows[c * per_core:(c + 1) * per_core]
        x_slices[c] = xp[nodes].T.astype(np.float16)

    # weight packs
    W1 = np.asarray(W1, np.float32)
    Bsrc = np.zeros((HEADS * HID, HEADS), np.float32)
    Bdst = np.zeros((HEADS * HID, HEADS), np.float32)
    for h in range(HEADS):
        Bsrc[h * HID:(h + 1) * HID, h] = np.asarray(a_src1[h], np.float32)
        Bdst[h * HID:(h + 1) * HID, h] = np.asarray(a_dst1[h], np.float32)
    W1big = np.concatenate([W1, W1 @ Bsrc, W1 @ Bdst], axis=1)  # [128, 136]
    asrc_rep = np.tile(W1big[0:1, 0:0], (128, 1))  # placeholder
    asrc_flat = np.asarray(a_src1, np.float32).reshape(1, HEADS * HID)
    asrc_rep = np.tile(asrc_flat, (128, 1))  # [128, 128] same row per partition
    W2 = np.asarray(W2, np.float32)
    W2big = np.concatenate(
        [W2, W2 @ np.asarray(a_src2, np.float32).T, W2 @ np.asarray(a_dst2, np.float32).T],
        axis=1,
    )  # [128, 34]
    b1v = np.asarray(b1, np.float32).reshape(HEADS, HID)
    b1i = np.zeros((HEADS, HID + 1), np.float32)
    b1i[:, :HID] = b1v
    b1_rep = np.tile(b1i.reshape(1, -1), (128, 1))                            # [128,132]
    b2_rep = np.zeros((128, HID + 2), np.float32)
    b2_rep[:, :HID] = np.asarray(b2, np.float32)[None, :]
    bout_rep = np.tile(np.asarray(b_out, np.float32)[None, :], (128, 1))     # [128,112]
    ident = np.eye(128, dtype=np.float32)

    bf16 = np.float16
    sent1 = np.zeros((1, T1_COLS), bf16)
    sent2 = np.zeros((1, 36), bf16)
    sent2[0, 32:34] = NEG_BIG

    meta = dict(
        N=N, E2=E2, n_pad=n_pad, per_core=per_core, NT=NT, nrows=nrows,
        b_base=BB, Ka=Ka.tolist(), Kb=Kb.tolist(),
        base_a=base_a.tolist(), base_b=base_b.tolist(), totidx=totidx,
        ga_base=ga_base, gb_base=gb_base, groups=groups,
        perm_rows=perm_rows,
    )
    shared = dict(
        W1big=W1big.astype(bf16), W2big=W2big.astype(bf16), asrc_rep=asrc_rep.astype(bf16),
        Wout=np.asarray(W_out, np.float32).astype(bf16),
        b1_rep=b1_rep, b2_rep=b2_rep, bout_rep=bout_rep, ident=ident.astype(bf16),
        sent1=sent1, sent2=sent2,
    )
    in_maps = []
    for c in range(N_CORES):
        m = dict(shared)
        m["x_slice"] = np.ascontiguousarray(x_slices[c])
        m["idx_flat"] = np.ascontiguousarray(idx_wrapped[c])
        m["edge_mask"] = np.ascontiguousarray(mask[c])
        in_maps.append(m)
    return meta, in_maps


def _build(meta):
    per_core, NT, nrows, b_base = meta["per_core"], meta["NT"], meta["nrows"], meta["b_base"]
    Ka, Kb = meta["Ka"], meta["Kb"]
    base_a, base_b, totidx = meta["base_a"], meta["base_b"], meta["totidx"]
    ga_base, gb_base, groups = meta["ga_base"], meta["gb_base"], meta["groups"]

    nc = bacc.Bacc("TRN2", num_devices=N_CORES, num_swdge_queues=4,
                   dynamic_dma_scratch_size=65536)

    x_slice = nc.dram_tensor("x_slice", [IN_CH, per_core], BF16, kind="ExternalInput")
    idx_flat = nc.dram_tensor("idx_flat", [128, totidx // 16], I16, kind="ExternalInput")
    W1big_d = nc.dram_tensor("W1big", [128, 136], BF16, kind="ExternalInput")
    asrc_d = nc.dram_tensor("asrc_rep", [128, 128], BF16, kind="ExternalInput")
    mask_d = nc.dram_tensor("edge_mask", [128, totidx // 128], BF16, kind="ExternalInput")
    W2big_d = nc.dram_tensor("W2big", [128, HID + 2], BF16, kind="ExternalInput")
    Wout_d = nc.dram_tensor("Wout", [HID, OUT_CH], BF16, kind="ExternalInput")
    b1_d = nc.dram_tensor("b1_rep", [128, 132], F32, kind="ExternalInput")
    b2_d = nc.dram_tensor("b2_rep", [128, HID + 2], F32, kind="ExternalInput")
    bout_d = nc.dram_tensor("bout_rep", [128, OUT_CH], F32, kind="ExternalInput")
    ident_d = nc.dram_tensor("ident", [128, 128], BF16, kind="ExternalInput")
    sent1_d = nc.dram_tensor("sent1", [1, T1_COLS], BF16, kind="ExternalInput")
    sent2_d = nc.dram_tensor("sent2", [1, 36], BF16, kind="ExternalInput")

    CHK = per_core // N_AG_CHUNKS
    TPC = NT // N_AG_CHUNKS  # tiles per AG chunk
    T1_ownc = [nc.dram_tensor(f"T1_own{j}", [CHK, T1_OWN], BF16, kind="Internal")
               for j in range(N_AG_CHUNKS)]
    T1_sh = nc.dram_tensor("T1_sh", [nrows, T1_COLS], BF16, kind="Internal", addr_space="Shared")
    T2_ownc = [nc.dram_tensor(f"T2_own{j}", [CHK, T2_OWN], BF16, kind="Internal")
               for j in range(N_AG_CHUNKS)]
    T2_sh = nc.dram_tensor("T2_sh", [nrows, T2_COLS], BF16, kind="Internal", addr_space="Shared")
    out_d = nc.dram_tensor("out", [per_core, OUT_CH], F32, kind="ExternalOutput")

    rgroups = [list(range(N_CORES))]
    qctr = [0]
    qsems = [nc.alloc_semaphore(f"gsem{q}") for q in range(4)]

    def qn():
        q = qctr[0] % 4
        qctr[0] += 1
        return q

    import os
    ASYNC = os.environ.get("GAT_ASYNC_GATHER", "0") == "1"

    def async_gather(out_ap, table_ap, idx_ap, nidx, cols):
        """Prep descriptors on a rotating SWDGE queue; fire the DMA async."""
        q = qn()
        if ASYNC:
            nc.gpsimd.dma_gather(
                out_ap, table_ap, idx_ap, nidx, nidx, cols,
                prepare_only=True, sem=qsems[q], queue_num=q, single_packet=False)
            nc.gpsimd.trigger_dma(count=None, queue_num=q)
        else:
            nc.gpsimd.dma_gather(
                out_ap, table_ap, idx_ap, nidx, nidx, cols,
                queue_num=q, single_packet=False)

    with tile.TileContext(nc) as tc:
        with (
            tc.tile_pool(name="const", bufs=1) as cp,
            tc.tile_pool(name="xa", bufs=2) as xap,
            tc.tile_pool(name="stage", bufs=3) as sp,
            tc.tile_pool(name="g1", bufs=3) as g1p,
            tc.tile_pool(name="g2", bufs=2) as g2p,
            tc.tile_pool(name="small", bufs=3) as smp,
            tc.tile_pool(name="rhs", bufs=2) as rp,
            tc.tile_pool(name="epi", bufs=2) as ep,
            tc.tile_pool(name="psa", bufs=3, space="PSUM") as ppa,
            tc.tile_pool(name="psm", bufs=3, space="PSUM") as ppm,
            tc.tile_pool(name="psy", bufs=2, space="PSUM") as ppy,
        ):
            # ---- consts to SBUF
            W1big = cp.tile([128, 136], BF16)
            nc.sync.dma_start(out=W1big[:], in_=W1big_d[:])
            asrc = cp.tile([128, 128], BF16)
            nc.sync.dma_start(out=asrc[:], in_=asrc_d[:])
            maskt = cp.tile([128, totidx // 128], BF16)
            nc.sync.dma_start(out=maskt[:], in_=mask_d[:])
            W2big = cp.tile([128, HID + 2], BF16)
            nc.sync.dma_start(out=W2big[:], in_=W2big_d[:])
            Wout = cp.tile([HID, OUT_CH], BF16)
            nc.sync.dma_start(out=Wout[:], in_=Wout_d[:])
            b1r = cp.tile([128, 132], F32)
            nc.sync.dma_start(out=b1r[:], in_=b1_d[:])
            b2r = cp.tile([128, HID + 2], F32)
            nc.sync.dma_start(out=b2r[:], in_=b2_d[:])
            boutr = cp.tile([128, OUT_CH], F32)
            nc.sync.dma_start(out=boutr[:], in_=bout_d[:])
            ident = cp.tile([128, 128], BF16)
            nc.sync.dma_start(out=ident[:], in_=ident_d[:])
            idxs = cp.tile([128, totidx // 16], I16)
            nc.sync.dma_start(out=idxs[:], in_=idx_flat[:])
            # SBUF-resident own-node data (no DRAM roundtrip)
            t1d_sb = cp.tile([128, NT * 8], BF16)    # [al_src(4)|al_dst(4)] per tile
            t2d_sb = cp.tile([128, NT * 2], BF16)    # [as2|ad2] per tile
            hball = cp.tile([128, NT * 132], BF16)   # own [h|1]x4 rows per tile
            nc.vector.memset(hball[:], 1.0)
            zball = cp.tile([128, NT * T2_USED], BF16)  # own [z|as2|ad2|1] per tile

            # ---- phase A: own node tiles -> T1_own
            for t in range(NT):
                xa = xap.tile([128, 128], BF16)
                nc.sync.dma_start(out=xa[:], in_=x_slice[:, t * 128:(t + 1) * 128])
                ps = ppa.tile([128, 136], F32, tag="agg")
                nc.tensor.matmul(out=ps[:], lhsT=xa[:], rhs=W1big[:], start=True, stop=True)
                hb = sp.tile([128, T1_COLS], BF16)
                nc.vector.tensor_copy(out=hb[:], in_=ps[:, 0:128])
                lt = (t % TPC) * 128
                nc.sync.dma_start(out=T1_ownc[t // TPC][lt:lt + 128, :], in_=hb[:])
                nc.scalar.activation(t1d_sb[:, t * 8:(t + 1) * 8], ps[:, 128:136],
                                     mybir.ActivationFunctionType.Copy)
                hball_v = hball[:, t * 132:(t + 1) * 132].rearrange("p (h j) -> p h j", h=HEADS)
                nc.scalar.activation(hball_v[:, :, 0:HID],
                                     ps[:, 0:128].rearrange("p (h c) -> p h c", h=HEADS),
                                     mybir.ActivationFunctionType.Copy)
                if t % TPC == TPC - 1:
                    j = t // TPC
                    o = 1 + j * CHK * N_CORES
                    nc.gpsimd.collective_compute(
                        "AllGather", mybir.AluOpType.bypass, replica_groups=rgroups,
                        ins=[T1_ownc[j][:]], outs=[T1_sh[o:o + CHK * N_CORES, :]],
                    )

            s1 = cp.tile([1, T1_COLS], BF16)
            nc.sync.dma_start(out=s1[:], in_=sent1_d[:])
            nc.sync.dma_start(out=T1_sh[0:1, :], in_=s1[:])
            nc.sync.dma_start(out=T1_sh[nrows - 1:nrows, :], in_=s1[:])

            # ---- layer 1 edge phase (grouped super-gathers)
            for gi, (t0, t1) in enumerate(groups):
                tiles = list(range(t0, t1))
                SA = sum(Ka[t] for t in tiles)
                SB = sum(Kb[t] for t in tiles)
                GA = g1p.tile([128, max(SA, 1), T1_COLS], BF16, tag="GA")
                if SA:
                    o = ga_base[gi]
                    async_gather(GA[:, 0:SA, :], T1_sh[:],
                                 idxs[:, o // 16: o // 16 + SA * 8], 128 * SA, T1_COLS)
                GB = g1p.tile([128, max(SB, 1), T1_COLS], BF16, tag="GB")
                if SB:
                    o = gb_base[gi]
                    async_gather(GB[:, 0:SB, :], T1_sh[b_base:nrows, :],
                                 idxs[:, o // 16: o // 16 + SB * 8], 128 * SB, T1_COLS)
                for t in tiles:
                    ka, kb = Ka[t], Kb[t]
                    kt = ka + kb
                    aoff = (base_a[t] - ga_base[gi]) // 128
                    boff = (base_b[t] - gb_base[gi]) // 128
                    ps = ppa.tile([128, HEADS * (HID + 1)], F32, tag="agg")
                    rhs = rp.tile([128, kt + 1, HEADS * (HID + 1)], BF16, tag="rhs1")
                    rhs_v = rhs[:].rearrange("p k (h j) -> p k h j", h=HEADS)
                    for (G, goff, nk, ro, mbase) in (
                            (GA, aoff, ka, 0, base_a[t] // 128),
                            (GB, boff, kb, ka, base_b[t] // 128)):
                        if nk == 0:
                            continue
                        Gs = G[:, goff:goff + nk, :]
                        Gh = Gs[:].rearrange("p k (h c) -> p k h c", h=HEADS)
                        # recompute al_src = sum_c h * a_src
                        tmp = smp.tile([128, nk, 128], BF16, tag="tmp")
                        nc.vector.tensor_tensor(
                            out=tmp[:], in0=Gs[:],
                            in1=asrc[:, None, :].to_broadcast([128, nk, 128]),
                            op=mybir.AluOpType.mult)
                        asl = smp.tile([128, nk, HEADS], F32, tag="asl")
                        nc.vector.tensor_reduce(
                            out=asl[:],
                            in_=tmp[:].rearrange("p k (h c) -> p k h c", h=HEADS),
                            axis=mybir.AxisListType.X, op=mybir.AluOpType.add)
                        xl = smp.tile([128, nk, HEADS], BF16, tag="xl")
                        nc.vector.tensor_tensor(
                            out=xl[:], in0=asl[:],
                            in1=t1d_sb[:, None, t * 8 + 4:t * 8 + 8].to_broadcast([128, nk, HEADS]),
                            op=mybir.AluOpType.add)
                        e1 = smp.tile([128, nk, HEADS], BF16, tag="e1")
                        nc.scalar.activation(e1[:], xl[:], mybir.ActivationFunctionType.Exp)
                        e2 = smp.tile([128, nk, HEADS], BF16, tag="e2")
                        nc.scalar.activation(e2[:], xl[:], mybir.ActivationFunctionType.Exp, scale=NEG_SLOPE)
                        wu = smp.tile([128, nk, HEADS], BF16, tag="wu")
                        nc.vector.tensor_tensor(out=wu[:], in0=e1[:], in1=e2[:], op=mybir.AluOpType.max)
                        # zero out pad slots
                        w = smp.tile([128, nk, HEADS], BF16, tag="w")
                        nc.vector.tensor_tensor(
                            out=w[:], in0=wu[:],
                            in1=maskt[:, mbase:mbase + nk, None].to_broadcast([128, nk, HEADS]),
                            op=mybir.AluOpType.mult)
                        nc.vector.tensor_tensor(
                            out=rhs_v[:, ro:ro + nk, :, 0:HID],
                            in0=Gh,
                            in1=w[:, :, :, None].to_broadcast([128, nk, HEADS, HID]),
                            op=mybir.AluOpType.mult)
                        nc.scalar.activation(rhs_v[:, ro:ro + nk, :, HID], w[:],
                                             mybir.ActivationFunctionType.Copy)
                    # self-loop chunk: w_self * [h|1]
                    xls = smp.tile([128, HEADS], BF16, tag="xls")
                    nc.vector.tensor_tensor(
                        out=xls[:], in0=t1d_sb[:, t * 8:t * 8 + 4],
                        in1=t1d_sb[:, t * 8 + 4:t * 8 + 8], op=mybir.AluOpType.add)
                    e1s = smp.tile([128, HEADS], BF16, tag="e1s")
                    nc.scalar.activation(e1s[:], xls[:], mybir.ActivationFunctionType.Exp)
                    e2s = smp.tile([128, HEADS], BF16, tag="e2s")
                    nc.scalar.activation(e2s[:], xls[:], mybir.ActivationFunctionType.Exp, scale=NEG_SLOPE)
                    ws = smp.tile([128, HEADS], BF16, tag="ws")
                    nc.vector.tensor_tensor(out=ws[:], in0=e1s[:], in1=e2s[:], op=mybir.AluOpType.max)
                    nc.vector.tensor_tensor(
                        out=rhs_v[:, kt, :, :],
                        in0=hball[:, t * 132:(t + 1) * 132].rearrange("p (h j) -> p h j", h=HEADS),
                        in1=ws[:, :, None].to_broadcast([128, HEADS, HID + 1]),
                        op=mybir.AluOpType.mult)
                    for cch in range(kt + 1):
                        nc.tensor.matmul(out=ps[:], lhsT=ident[:], rhs=rhs[:, cch, :],
                                         start=(cch == 0), stop=(cch == kt))
                    # epilogue: divide, +b1, ELU
                    ps_v = ps[:].rearrange("p (h j) -> p h j", h=HEADS)
                    rec = smp.tile([128, HEADS], F32, tag="rec")
                    nc.vector.reciprocal(out=rec[:], in_=ps_v[:, :, HID])
                    y1 = ppy.tile([128, 128], F32, tag="y")
                    nc.vector.tensor_tensor(
                        out=y1[:].rearrange("p (h j) -> p h j", h=HEADS),
                        in0=ps_v[:, :, 0:HID],
                        in1=rec[:, :, None].to_broadcast([128, HEADS, HID]),
                        op=mybir.AluOpType.mult)
                    nc.vector.tensor_tensor(
                        out=y1[:].rearrange("p (h j) -> p h j", h=HEADS),
                        in0=y1[:].rearrange("p (h j) -> p h j", h=HEADS),
                        in1=b1r[:].rearrange("p (h j) -> p h j", h=HEADS)[:, :, 0:HID],
                        op=mybir.AluOpType.add)
                    m1 = ep.tile([128, 128], F32, tag="m1")
                    nc.vector.tensor_scalar(out=m1[:], in0=y1[:], scalar1=0.0, scalar2=None,
                                            op0=mybir.AluOpType.min)
                    eE = ep.tile([128, 128], F32, tag="eE")
                    nc.scalar.activation(eE[:], m1[:], mybir.ActivationFunctionType.Exp)
                    r1 = ep.tile([128, 128], F32, tag="r1")
                    nc.vector.tensor_scalar(out=r1[:], in0=y1[:], scalar1=0.0, scalar2=-1.0,
                                            op0=mybir.AluOpType.max, op1=mybir.AluOpType.add)
                    h2 = ep.tile([128, 128], BF16, tag="h2")
                    nc.vector.tensor_tensor(out=h2[:], in0=eE[:], in1=r1[:], op=mybir.AluOpType.add)
                    # transpose h2, z = h2 @ W2big
                    pt = ppm.tile([128, 128], BF16, tag="misc")
                    nc.tensor.transpose(out=pt[:], in_=h2[:], identity=ident[:])
                    h2T = ep.tile([128, 128], BF16, tag="h2T")
                    nc.scalar.activation(h2T[:], pt[:], mybir.ActivationFunctionType.Copy)
                    psz = ppm.tile([128, HID + 2], F32, tag="misc")
                    nc.tensor.matmul(out=psz[:], lhsT=h2T[:], rhs=W2big[:], start=True, stop=True)
                    t2b = sp.tile([128, T2_OWN], BF16, tag="t2b")
                    nc.vector.tensor_tensor(out=t2b[:, 0:HID + 2], in0=psz[:], in1=b2r[:],
                                            op=mybir.AluOpType.add)
                    nc.vector.memset(t2b[:, HID + 2:HID + 3], 1.0)
                    lt2 = (t % TPC) * 128
                    nc.sync.dma_start(out=T2_ownc[t // TPC][lt2:lt2 + 128, :], in_=t2b[:])
                    nc.scalar.activation(zball[:, t * T2_USED:(t + 1) * T2_USED],
                                         t2b[:, 0:T2_USED], mybir.ActivationFunctionType.Copy)
                    nc.scalar.activation(t2d_sb[:, t * 2:(t + 1) * 2], t2b[:, 32:34],
                                         mybir.ActivationFunctionType.Copy)
                    if t % TPC == TPC - 1:
                        j = t // TPC
                        o = 1 + j * CHK * N_CORES
                        nc.gpsimd.collective_compute(
                            "AllGather", mybir.AluOpType.bypass, replica_groups=rgroups,
                            ins=[T2_ownc[j][:]], outs=[T2_sh[o:o + CHK * N_CORES, :]],
                        )

            # ---- T2 sentinel pokes
            s2 = cp.tile([1, 36], BF16)
            nc.sync.dma_start(out=s2[:], in_=sent2_d[:])
            nc.sync.dma_start(out=T2_sh[0:1, 0:36], in_=s2[:])
            nc.sync.dma_start(out=T2_sh[nrows - 1:nrows, 0:36], in_=s2[:])

            # ---- layer 2 edge phase + output (grouped super-gathers)
            for gi, (t0, t1) in enumerate(groups):
                tiles2 = list(range(t0, t1))
                SA = sum(Ka[t] for t in tiles2)
                SB = sum(Kb[t] for t in tiles2)
                GA2 = g2p.tile([128, max(SA, 1), T2_COLS], BF16, tag="GA2")
                if SA:
                    o = ga_base[gi]
                    async_gather(GA2[:, 0:SA, :], T2_sh[:],
                                 idxs[:, o // 16: o // 16 + SA * 8], 128 * SA, T2_COLS)
                GB2 = g2p.tile([128, max(SB, 1), T2_COLS], BF16, tag="GB2")
                if SB:
                    o = gb_base[gi]
                    async_gather(GB2[:, 0:SB, :], T2_sh[b_base:nrows, :],
                                 idxs[:, o // 16: o // 16 + SB * 8], 128 * SB, T2_COLS)
                for t in tiles2:
                    ka, kb = Ka[t], Kb[t]
                    kt = ka + kb
                    aoff = (base_a[t] - ga_base[gi]) // 128
                    boff = (base_b[t] - gb_base[gi]) // 128
                    ps2 = ppa.tile([128, T2_USED], F32, tag="agg")
                    rhs2 = rp.tile([128, kt + 1, T2_USED], BF16, tag="rhs2")
                    for (G2, goff, nk, ro) in ((GA2, aoff, ka, 0), (GB2, boff, kb, ka)):
                        if nk == 0:
                            continue
                        Gs = G2[:, goff:goff + nk, :]
                        xl2 = smp.tile([128, nk, 1], BF16, tag="xl2")
                        nc.vector.tensor_tensor(
                            out=xl2[:], in0=Gs[:, :, 32:33],
                            in1=t2d_sb[:, None, t * 2 + 1:t * 2 + 2].to_broadcast([128, nk, 1]),
                            op=mybir.AluOpType.add)
                        e1b = smp.tile([128, nk, 1], BF16, tag="e1b")
                        nc.scalar.activation(e1b[:], xl2[:], mybir.ActivationFunctionType.Exp)
                        e2b = smp.tile([128, nk, 1], BF16, tag="e2b")
                        nc.scalar.activation(e2b[:], xl2[:], mybir.ActivationFunctionType.Exp, scale=NEG_SLOPE)
                        w2 = smp.tile([128, nk, 1], BF16, tag="w2")
                        nc.vector.tensor_tensor(out=w2[:], in0=e1b[:], in1=e2b[:], op=mybir.AluOpType.max)
                        nc.vector.tensor_tensor(
                            out=rhs2[:, ro:ro + nk, :],
                            in0=Gs[:, :, 0:T2_USED],
                            in1=w2[:, :, :].to_broadcast([128, nk, T2_USED]),
                            op=mybir.AluOpType.mult)
                    # self-loop chunk
                    xl2s = smp.tile([128, 1], BF16, tag="xl2s")
                    nc.vector.tensor_tensor(
                        out=xl2s[:], in0=t2d_sb[:, t * 2:t * 2 + 1],
                        in1=t2d_sb[:, t * 2 + 1:t * 2 + 2], op=mybir.AluOpType.add)
                    e1bs = smp.tile([128, 1], BF16, tag="e1bs")
                    nc.scalar.activation(e1bs[:], xl2s[:], mybir.ActivationFunctionType.Exp)
                    e2bs = smp.tile([128, 1], BF16, tag="e2bs")
                    nc.scalar.activation(e2bs[:], xl2s[:], mybir.ActivationFunctionType.Exp, scale=NEG_SLOPE)
                    w2s = smp.tile([128, 1], BF16, tag="w2s")
                    nc.vector.tensor_tensor(out=w2s[:], in0=e1bs[:], in1=e2bs[:], op=mybir.AluOpType.max)
                    nc.vector.tensor_tensor(
                        out=rhs2[:, kt, :],
                        in0=zball[:, t * T2_USED:(t + 1) * T2_USED],
                        in1=w2s[:].to_broadcast([128, T2_USED]),
                        op=mybir.AluOpType.mult)
                    for cch in range(kt + 1):
                        nc.tensor.matmul(out=ps2[:], lhsT=ident[:], rhs=rhs2[:, cch, :],
                                         start=(cch == 0), stop=(cch == kt))
                    rec2 = smp.tile([128, 1], F32, tag="rec2")
                    nc.vector.reciprocal(out=rec2[:], in_=ps2[:, HID + 2:HID + 3])
                    y2 = ppy.tile([128, HID], F32, tag="y")
                    nc.vector.tensor_tensor(
                        out=y2[:], in0=ps2[:, 0:HID],
                        in1=rec2[:].to_broadcast([128, HID]),
                        op=mybir.AluOpType.mult)
                    m2 = ep.tile([128, HID], F32, tag="m2")
                    nc.vector.tensor_scalar(out=m2[:], in0=y2[:], scalar1=0.0, scalar2=None,
                                            op0=mybir.AluOpType.min)
                    eE2 = ep.tile([128, HID], F32, tag="eE2")
                    nc.scalar.activation(eE2[:], m2[:], mybir.ActivationFunctionType.Exp)
                    r2 = ep.tile([128, HID], F32, tag="r2")
                    nc.vector.tensor_scalar(out=r2[:], in0=y2[:], scalar1=0.0, scalar2=-1.0,
                                            op0=mybir.AluOpType.max, op1=mybir.AluOpType.add)
                    h3 = ep.tile([128, HID], BF16, tag="h3")
                    nc.vector.tensor_tensor(out=h3[:], in0=eE2[:], in1=r2[:], op=mybir.AluOpType.add)
                    pt2 = ppm.tile([128, 128], BF16, tag="misc")
                    nc.tensor.transpose(out=pt2[:HID, :], in_=h3[:], identity=ident[:])
                    h3T = ep.tile([HID, 128], BF16, tag="h3T")
                    nc.scalar.activation(h3T[:], pt2[:HID, :], mybir.ActivationFunctionType.Copy)
                    psf = ppm.tile([128, OUT_CH], F32, tag="misc")
                    nc.tensor.matmul(out=psf[:], lhsT=h3T[:], rhs=Wout[:], start=True, stop=True)
                    outf = ep.tile([128, OUT_CH], F32, tag="outf")
                    nc.vector.tensor_tensor(out=outf[:], in0=psf[:], in1=boutr[:],
                                            op=mybir.AluOpType.add)
                    nc.sync.dma_start(out=out_d[t * 128:(t + 1) * 128, :], in_=outf[:])

    nc.compile()
    return nc


def _run(inputs, trace=False):
    meta, in_maps = _prep(**inputs)
    nc = _build(meta)
    res = run_bass_kernel_spmd(nc, in_maps, core_ids=list(range(N_CORES)), trace=trace)
    outg = np.concatenate([res.results[c]["out"] for c in range(N_CORES)], axis=0)
    out_nodes = np.empty((meta["n_pad"], OUT_CH), np.float32)
    out_nodes[meta["perm_rows"]] = outg
    return out_nodes[:meta["N"]], res


def kernel(**inputs):
    out, _ = _run(inputs, trace=False)
    return out
